# revision 19
# baseline (speedup 1.0000x reference)
"""Trainium2 Bass kernel for chunked flash-attention block (B=2, S=2048, D=1024, H=16).

Sharding: 8 cores = 2 batches x 4 head-groups (4 heads each). Each core computes
its heads' QKV projections + RoPE + per-chunk-softmax attention + its slice of the
output projection; the host sums the 4 partial out-projections per batch.

All device-side activations are kept transposed ([feature, seq]) so every matmul
contraction lands on the partition dimension with no on-device transposes of the
inputs. Emission order streams the attention (exp on ScalarE is the bottleneck):
head-pair 0 attention starts right after its projections; head-pair 1 projections
are interleaved into head-pair 0's attention; the output projection is fused into
the last attention pass.
"""

import numpy as np
import ml_dtypes

import concourse.bass as bass
import concourse.tile as tile
from concourse import bacc, mybir
from concourse.bass_utils import run_bass_kernel_spmd
from concourse.masks import make_identity

dt = mybir.dt
F32 = dt.float32
BF16 = dt.bfloat16
AF = mybir.ActivationFunctionType
OP = mybir.AluOpType

B, S, D, H, HD = 2, 2048, 1024, 16, 64
CHUNK = 1024
NHL = 4              # local heads per core
JL = NHL * HD        # 256 local projected dims
ND = D // 128        # 8 k-tiles for the projections
NSQ = S // 512       # 4 sq n-tiles
NSK = S // 128       # 16 sk p-tiles
NCH = S // CHUNK     # 2 key chunks
TPC = CHUNK // 128   # 8 sk tiles per chunk
NM = S // 128        # 16 sq p-tiles

# exp batches per (hp, chunk, n): 16 (t, h) score slots -> 6 ACT instructions
SC_BATCHES = ((0, 3), (3, 3), (6, 3), (9, 3), (12, 2), (14, 2))

_CACHED = {}


def _emit_body(nc, tc, persist, rope, aps, rep):
    """Emit one full iteration of the kernel into the open TileContext."""
    xT_d, wq_d, wk_d, wv_d, wo_d, c2_d, s2_d, out_d = aps
    r = f"r{rep}"

    # DMA order matters: xT + wq gate the first projection matmuls.
    xsb = persist.tile([128, ND, S], BF16, tag="xT", name=f"xT_{r}")
    xT_r = xT_d.rearrange("(t p) s -> p t s", p=128)
    for di in range(ND):
        nc.sync.dma_start(xsb[:, di, :], xT_r[:, di, :])
    wsbs = {}
    for nm, d_ap in (("wq", wq_d), ("wk", wk_d), ("wv", wv_d)):
        wsbs[nm] = persist.tile([128, ND, JL], BF16, tag=nm, name=f"{nm}_{r}")
        nc.sync.dma_start(wsbs[nm][:], d_ap.rearrange("(t p) j -> p t j", p=128))
    c2 = persist.tile([128, S], BF16, tag="c2", name=f"c2_{r}")
    s2 = persist.tile([128, S], BF16, tag="s2", name=f"s2_{r}")
    nc.sync.dma_start(c2[:], c2_d)
    nc.sync.dma_start(s2[:], s2_d)
    wo_sb = persist.tile([128, 2, D], BF16, tag="wo", name=f"wo_{r}")
    nc.sync.dma_start(wo_sb[:], wo_d.rearrange("(t p) n -> p t n", p=128))
    ident = persist.tile([128, 128], BF16, tag="ident", name=f"ident_{r}")
    make_identity(nc, ident[:])

    qTr = persist.tile([128, 2, S], BF16, tag="qTr", name=f"qTr_{r}")
    kTr = persist.tile([128, 2, S], BF16, tag="kTr", name=f"kTr_{r}")
    vON = persist.tile([128, NSK, NHL * 65], BF16, tag="vON", name=f"vON_{r}")
    attn = persist.tile([128, NM, JL], BF16, tag="attn", name=f"attn_{r}")
    attnT = persist.tile([128, 2, S], BF16, tag="attnT", name=f"attnT_{r}")

    vON_r = vON[:].rearrange("p t (h c) -> p (t h) c", c=65)
    nc.vector.memset(vON_r[:, :, 64:65], 1.0)

    def emit_qk_tile(pool, wsb, dst, jt, st):
        """One [128, 512] q/k projection tile + RoPE into dst (qTr/kTr)."""
        sl = slice(st * 512, (st + 1) * 512)
        ps = pool.tile([128, 512], F32, tag="ps1", name=f"pqk_{r}")
        for di in range(ND):
            nc.tensor.matmul(
                ps[:],
                lhsT=wsb[:, di, jt * 128:(jt + 1) * 128],
                rhs=xsb[:, di, sl],
                start=(di == 0),
                stop=(di == ND - 1),
            )
        # RoPE in bf16 (fast DVE mode). rot[p] = q[p]*cos - (q*s2)[swap(p)]
        # where swap flips the a/b 32-row halves within each head; the
        # partition swap rides on GpSimd (single-input shifted copies).
        qb = rope.tile([128, 512], BF16, tag="qb", name=f"qb_{r}")
        nc.vector.tensor_copy(qb[:], ps[:])
        w2 = rope.tile([128, 512], BF16, tag="w2", name=f"w2_{r}")
        nc.vector.tensor_mul(w2[:], qb[:], s2[:, sl])
        u = rope.tile([128, 512], BF16, tag="u", name=f"u_{r}")
        for blk in range(4):
            o = blk * 32
            so = o ^ 32
            eng = nc.gpsimd if blk < 2 else nc.vector
            eng.tensor_copy(u[o:o + 32, :], w2[so:so + 32, :])
        t2 = rope.tile([128, 512], BF16, tag="t2", name=f"t2_{r}")
        nc.vector.tensor_mul(t2[:], qb[:], c2[:, sl])
        nc.vector.tensor_sub(dst[:, jt, sl], t2[:], u[:])

    def emit_v_tile(pool, st):
        psv = pool.tile([128, JL], F32, tag="ps1", name=f"pv_{r}")
        for di in range(ND):
            nc.tensor.matmul(
                psv[:],
                lhsT=xsb[:, di, st * 128:(st + 1) * 128],
                rhs=wsbs["wv"][:, di, :],
                start=(di == 0),
                stop=(di == ND - 1),
            )
        nc.vector.tensor_copy(
            vON_r[:, st * NHL:(st + 1) * NHL, 0:64],
            psv[:].rearrange("p (h e) -> p h e", e=64),
        )

    # ---------------- attention (projections stream through the same pool) --
    with (
        tc.tile_pool(name=f"sc_{r}", bufs=2, space="PSUM") as scp,
        tc.tile_pool(name=f"ps1_{r}", bufs=2, space="PSUM") as ps1p,
        tc.tile_pool(name=f"expp_{r}", bufs=3) as expp,
        tc.tile_pool(name=f"normp_{r}", bufs=8) as normp,
        tc.tile_pool(name=f"osb_{r}", bufs=4) as osbp,
    ):
        # phase 1: jt0 projections only — attention for head-pair 0 starts
        # right after these; everything else streams in between exp batches.
        # Order so the first scores' deps (q st0, k st0-1) complete earliest.
        for w, dst, st in (("wq", qTr, 0), ("wk", kTr, 0), ("wk", kTr, 1),
                           ("wq", qTr, 1), ("wq", qTr, 2), ("wq", qTr, 3),
                           ("wk", kTr, 2), ("wk", kTr, 3)):
            emit_qk_tile(ps1p, wsbs[w], dst, 0, st)
        def emit_scores(hp, c, n, exp_tiles):
            et = expp.tile([128, 16, 512], BF16, tag="expT", name=f"expT_{r}")
            exp_tiles[n] = et
            for start, size in SC_BATCHES:
                sc = scp.tile([128, size, 512], F32, tag="sc", name=f"sc_{r}")
                for k in range(size):
                    slot = start + k
                    t2_, h = slot // 2, slot % 2
                    tg = c * TPC + t2_
                    nc.tensor.matmul(
                        sc[:, k, :],
                        lhsT=kTr[64 * h:64 * (h + 1), hp, tg * 128:(tg + 1) * 128],
                        rhs=qTr[64 * h:64 * (h + 1), hp, n * 512:(n + 1) * 512],
                        start=True, stop=True,
                    )
                nc.scalar.activation(
                    out=et[:, start:start + size, :],
                    in_=sc[:, 0:size, :],
                    func=AF.Exp,
                )

        def emit_wv(hp, c, n, exp_tiles):
            et = exp_tiles.pop(n)
            for m2 in range(4):
                m = n * 4 + m2
                # both heads packed in one PSUM bank: head h at cols [66h, 66h+65)
                psB = ps1p.tile([128, 132], F32, tag="ps1", name=f"psB_{r}")
                psBr = psB[:].rearrange("p (h e) -> p h e", e=66)
                for h in range(2):
                    hl = hp * 2 + h
                    for t2_ in range(TPC):
                        tg = c * TPC + t2_
                        nc.tensor.matmul(
                            psB[:, h * 66:h * 66 + 65],
                            lhsT=et[:, 2 * t2_ + h, m2 * 128:(m2 + 1) * 128],
                            rhs=vON[:, tg, hl * 65:(hl + 1) * 65],
                            start=(h == 0 and t2_ == 0),
                            stop=(h == 1 and t2_ == TPC - 1),
                            skip_group_check=True,
                        )
                rec = normp.tile([128, 2], F32, tag="rec", name=f"rec_{r}")
                nc.vector.reciprocal(rec[:, :], psBr[:, :, 64:65])
                for h in range(2):
                    hl = hp * 2 + h
                    dstp = attn[:, m, hl * 64:(hl + 1) * 64]
                    if c == 0:
                        nc.vector.tensor_scalar_mul(dstp, psBr[:, h, 0:64], rec[:, h:h + 1])
                    else:
                        nc.vector.scalar_tensor_tensor(
                            out=dstp, in0=psBr[:, h, 0:64], scalar=rec[:, h:h + 1],
                            in1=dstp, op0=OP.mult, op1=OP.add,
                        )

        def emit_out_quarter(n):
            """Transpose + output projection for sq tiles m in quarter n."""
            for m in range(n * 4, n * 4 + 4):
                for jt in range(2):
                    pstile = ps1p.tile([128, 128], BF16, tag="ps1", name=f"pt_{r}")
                    nc.tensor.transpose(pstile[:], attn[:, m, jt * 128:(jt + 1) * 128], ident[:])
                    nc.vector.tensor_copy(attnT[:, jt, m * 128:(m + 1) * 128], pstile[:])
            for m in range(n * 4, n * 4 + 4):
                for nn in range(2):
                    pso = ps1p.tile([128, 512], F32, tag="ps1", name=f"po_{r}")
                    for jt in range(2):
                        nc.tensor.matmul(
                            pso[:],
                            lhsT=attnT[:, jt, m * 128:(m + 1) * 128],
                            rhs=wo_sb[:, jt, nn * 512:(nn + 1) * 512],
                            start=(jt == 0),
                            stop=(jt == 1),
                        )
                    osb = osbp.tile([128, 512], BF16, tag="osb", name=f"osb_{r}")
                    if (m + nn) % 2 == 0:
                        nc.scalar.copy(osb[:], pso[:])
                    else:
                        nc.vector.tensor_copy(osb[:], pso[:])
                    nc.sync.dma_start(out_d[m * 128:(m + 1) * 128, nn * 512:(nn + 1) * 512], osb[:])

        # per-(hp, c) list of "extra" work items interleaved between exp
        # batches: v-projection tiles, deferred jt1 projections, fused
        # output-projection quarters. Slot positions: after wv(0..3).
        def qk(w, dst, jt, st):
            return lambda: emit_qk_tile(ps1p, wsbs[w], dst, jt, st)

        def vt(st):
            return lambda: emit_v_tile(ps1p, st)

        extras = {
            (0, 0): [
                [qk("wq", qTr, 1, 0)],
                [qk("wq", qTr, 1, 1), qk("wq", qTr, 1, 2)],
                [qk("wq", qTr, 1, 3)],
                [],
            ],
            (0, 1): [
                [qk("wk", kTr, 1, 0), qk("wk", kTr, 1, 1)],
                [qk("wk", kTr, 1, 2)],
                [qk("wk", kTr, 1, 3)],
                [],
            ],
            (1, 0): [[], [], [], []],
            (1, 1): [[], [], [], []],
        }

        for hp in range(2):
            for c in range(2):
                last = (hp == 1 and c == 1)
                ext = extras[(hp, c)]
                ex = {}
                # v tiles needed by this (hp, c)'s wv: chunk tiles c*8..c*8+8
                pre = [vt(c * TPC + i) for i in range(TPC)] if hp == 0 else []
                emit_scores(hp, c, 0, ex)
                emit_scores(hp, c, 1, ex)
                for f in pre:
                    f()
                emit_wv(hp, c, 0, ex)
                for f in ext[0]:
                    f()
                if last:
                    emit_out_quarter(0)
                emit_scores(hp, c, 2, ex)
                emit_wv(hp, c, 1, ex)
                for f in ext[1]:
                    f()
                if last:
                    emit_out_quarter(1)
                emit_scores(hp, c, 3, ex)
                emit_wv(hp, c, 2, ex)
                for f in ext[2]:
                    f()
                if last:
                    emit_out_quarter(2)
                emit_wv(hp, c, 3, ex)
                for f in ext[3]:
                    f()
                if last:
                    emit_out_quarter(3)


def _build_nc(reps=1):
    nc = bacc.Bacc("TRN2", target_bir_lowering=False, debug=False, num_devices=8)

    aps = (
        nc.dram_tensor("xT", [D, S], BF16, kind="ExternalInput").ap(),
        nc.dram_tensor("wq", [D, JL], BF16, kind="ExternalInput").ap(),
        nc.dram_tensor("wk", [D, JL], BF16, kind="ExternalInput").ap(),
        nc.dram_tensor("wv", [D, JL], BF16, kind="ExternalInput").ap(),
        nc.dram_tensor("wo", [JL, D], BF16, kind="ExternalInput").ap(),
        nc.dram_tensor("c2", [128, S], BF16, kind="ExternalInput").ap(),
        nc.dram_tensor("s2", [128, S], BF16, kind="ExternalInput").ap(),
        nc.dram_tensor("out", [S, D], BF16, kind="ExternalOutput").ap(),
    )

    with (
        tile.TileContext(nc) as tc,
        tc.tile_pool(name="persist", bufs=1) as persist,
        tc.tile_pool(name="rope", bufs=3) as rope,
    ):
        for rep in range(reps):
            _emit_body(nc, tc, persist, rope, aps, rep)

    nc.compile()
    return nc


def _get_nc(reps=1):
    if reps not in _CACHED:
        _CACHED[reps] = _build_nc(reps)
    return _CACHED[reps]


def _host_prep(hidden_states, freqs_cis, Wq, Wk, Wv, Wo):
    bf16 = ml_dtypes.bfloat16
    hs = np.asarray(hidden_states, dtype=np.float32)
    fc = np.asarray(freqs_cis, dtype=np.float32)
    Wq = np.asarray(Wq, dtype=np.float32)
    Wk = np.asarray(Wk, dtype=np.float32)
    Wv = np.asarray(Wv, dtype=np.float32)
    Wo = np.asarray(Wo, dtype=np.float32)

    cos, sin = fc[:, :, 0], fc[:, :, 1]                      # [S, 32]
    i_idx = np.arange(128) % 32
    sign = np.where((np.arange(128) % 64) < 32, -1.0, 1.0).astype(np.float32)
    c2 = np.ascontiguousarray(cos.T[i_idx]).astype(bf16)     # [128, S]
    s2 = np.ascontiguousarray(sin.T[i_idx] * sign[:, None]).astype(bf16)

    xTs = [np.ascontiguousarray(hs[b].T).astype(bf16) for b in range(B)]

    in_maps = []
    for core in range(8):
        b, g = core // 4, core % 4
        perm = []
        for h in range(4 * g, 4 * g + 4):
            perm += [h * 64 + 2 * i for i in range(32)]
            perm += [h * 64 + 2 * i + 1 for i in range(32)]
        perm = np.array(perm)
        jcols = slice(g * JL, (g + 1) * JL)
        in_maps.append({
            "xT": xTs[b],
            "wq": np.ascontiguousarray(Wq[:, perm] * (HD ** -0.5)).astype(bf16),
            "wk": np.ascontiguousarray(Wk[:, perm]).astype(bf16),
            "wv": np.ascontiguousarray(Wv[:, jcols]).astype(bf16),
            "wo": np.ascontiguousarray(Wo[jcols, :]).astype(bf16),
            "c2": c2,
            "s2": s2,
        })
    return in_maps


def kernel(hidden_states, freqs_cis, Wq, Wk, Wv, Wo, _trace=False, _reps=1):
    nc = _get_nc(_reps)
    in_maps = _host_prep(hidden_states, freqs_cis, Wq, Wk, Wv, Wo)
    if _trace:
        try:
            from antenv.axon_hooks import get_axon_ntff_profile_hook  # noqa: F401
        except ImportError:
            _trace = False
    res = run_bass_kernel_spmd(nc, in_maps, core_ids=list(range(8)), trace=_trace)
    outs = [r["out"].astype(np.float32) for r in res.results]
    full = np.zeros((B, S, D), dtype=np.float32)
    for core in range(8):
        full[core // 4] += outs[core]
    if _trace:
        kernel._last_results = res
    return full


# revision 21
# speedup vs baseline: 1515.3319x; 1515.3319x over previous
"""Trainium2 Bass kernel for chunked flash-attention block (B=2, S=2048, D=1024, H=16).

Sharding: 8 cores = 2 batches x 4 head-groups (4 heads each). Each core computes
its heads' QKV projections + RoPE + per-chunk-softmax attention + its slice of the
output projection; the host sums the 4 partial out-projections per batch.

All device-side activations are kept transposed ([feature, seq]) so every matmul
contraction lands on the partition dimension with no on-device transposes of the
inputs. Emission order streams the attention (exp on ScalarE is the bottleneck):
head-pair 0 attention starts right after its projections; head-pair 1 projections
are interleaved into head-pair 0's attention; the output projection is fused into
the last attention pass.
"""

import numpy as np
import ml_dtypes

import concourse.bass as bass
import concourse.tile as tile
from concourse import bacc, mybir
from concourse.bass_utils import run_bass_kernel_spmd
from concourse.masks import make_identity

dt = mybir.dt
F32 = dt.float32
BF16 = dt.bfloat16
AF = mybir.ActivationFunctionType
OP = mybir.AluOpType

B, S, D, H, HD = 2, 2048, 1024, 16, 64
CHUNK = 1024
NHL = 4              # local heads per core
JL = NHL * HD        # 256 local projected dims
ND = D // 128        # 8 k-tiles for the projections
NSQ = S // 512       # 4 sq n-tiles
NSK = S // 128       # 16 sk p-tiles
NCH = S // CHUNK     # 2 key chunks
TPC = CHUNK // 128   # 8 sk tiles per chunk
NM = S // 128        # 16 sq p-tiles

# exp batches per (hp, chunk, n): 16 (t, h) score slots -> 6 ACT instructions
SC_BATCHES = ((0, 3), (3, 3), (6, 3), (9, 3), (12, 2), (14, 2))

_CACHED = {}


def _emit_body(nc, tc, persist, rope, aps, rep):
    """Emit one full iteration of the kernel into the open TileContext."""
    xT_d, wq_d, wk_d, wv_d, wo_d, c2_d, s2_d, out_d = aps
    r = f"r{rep}"

    # DMA order matters: xT + wq gate the first projection matmuls.
    xsb = persist.tile([128, ND, S], BF16, tag="xT", name=f"xT_{r}")
    xT_r = xT_d.rearrange("(t p) s -> p t s", p=128)
    for di in range(ND):
        nc.sync.dma_start(xsb[:, di, :], xT_r[:, di, :])
    wsbs = {}
    for nm, d_ap in (("wq", wq_d), ("wk", wk_d), ("wv", wv_d)):
        wsbs[nm] = persist.tile([128, ND, JL], BF16, tag=nm, name=f"{nm}_{r}")
        nc.sync.dma_start(wsbs[nm][:], d_ap.rearrange("(t p) j -> p t j", p=128))
    c2 = persist.tile([128, S], BF16, tag="c2", name=f"c2_{r}")
    s2 = persist.tile([128, S], BF16, tag="s2", name=f"s2_{r}")
    nc.sync.dma_start(c2[:], c2_d)
    nc.sync.dma_start(s2[:], s2_d)
    wo_sb = persist.tile([128, 2, D], BF16, tag="wo", name=f"wo_{r}")
    nc.sync.dma_start(wo_sb[:], wo_d.rearrange("(t p) n -> p t n", p=128))
    ident = persist.tile([128, 128], BF16, tag="ident", name=f"ident_{r}")
    make_identity(nc, ident[:])

    qTr = persist.tile([128, 2, S], BF16, tag="qTr", name=f"qTr_{r}")
    kTr = persist.tile([128, 2, S], BF16, tag="kTr", name=f"kTr_{r}")
    vON = persist.tile([128, NSK, NHL * 65], BF16, tag="vON", name=f"vON_{r}")
    attn = persist.tile([128, NM, JL], BF16, tag="attn", name=f"attn_{r}")
    attnT = persist.tile([128, 2, S], BF16, tag="attnT", name=f"attnT_{r}")

    vON_r = vON[:].rearrange("p t (h c) -> p (t h) c", c=65)
    nc.vector.memset(vON_r[:, :, 64:65], 1.0)

    def emit_qk_tile(pool, wsb, dst, jt, st):
        """One [128, 512] q/k projection tile + RoPE into dst (qTr/kTr)."""
        sl = slice(st * 512, (st + 1) * 512)
        ps = pool.tile([128, 512], F32, tag="ps1", name=f"pqk_{r}")
        for di in range(ND):
            nc.tensor.matmul(
                ps[:],
                lhsT=wsb[:, di, jt * 128:(jt + 1) * 128],
                rhs=xsb[:, di, sl],
                start=(di == 0),
                stop=(di == ND - 1),
            )
        # RoPE in bf16 (fast DVE mode). rot[p] = q[p]*cos - (q*s2)[swap(p)]
        # where swap flips the a/b 32-row halves within each head; the
        # partition swap rides on GpSimd (single-input shifted copies).
        qb = rope.tile([128, 512], BF16, tag="qb", name=f"qb_{r}")
        nc.vector.tensor_copy(qb[:], ps[:])
        w2 = rope.tile([128, 512], BF16, tag="w2", name=f"w2_{r}")
        nc.vector.tensor_mul(w2[:], qb[:], s2[:, sl])
        u = rope.tile([128, 512], BF16, tag="u", name=f"u_{r}")
        for blk in range(4):
            o = blk * 32
            so = o ^ 32
            eng = nc.gpsimd if blk < 2 else nc.vector
            eng.tensor_copy(u[o:o + 32, :], w2[so:so + 32, :])
        t2 = rope.tile([128, 512], BF16, tag="t2", name=f"t2_{r}")
        nc.vector.tensor_mul(t2[:], qb[:], c2[:, sl])
        nc.vector.tensor_sub(dst[:, jt, sl], t2[:], u[:])

    def emit_v_tile(pool, st):
        psv = pool.tile([128, JL], F32, tag="ps1", name=f"pv_{r}")
        for di in range(ND):
            nc.tensor.matmul(
                psv[:],
                lhsT=xsb[:, di, st * 128:(st + 1) * 128],
                rhs=wsbs["wv"][:, di, :],
                start=(di == 0),
                stop=(di == ND - 1),
            )
        nc.vector.tensor_copy(
            vON_r[:, st * NHL:(st + 1) * NHL, 0:64],
            psv[:].rearrange("p (h e) -> p h e", e=64),
        )

    # ---------------- attention (projections stream through the same pool) --
    with (
        tc.tile_pool(name=f"sc_{r}", bufs=2, space="PSUM") as scp,
        tc.tile_pool(name=f"ps1_{r}", bufs=2, space="PSUM") as ps1p,
        tc.tile_pool(name=f"expp_{r}", bufs=3) as expp,
        tc.tile_pool(name=f"normp_{r}", bufs=8) as normp,
        tc.tile_pool(name=f"osb_{r}", bufs=4) as osbp,
    ):
        # phase 1: only the projections gating the very first score batch —
        # PE executes in emission order, so everything else streams between
        # score/exp groups (each projection must still be emitted before the
        # first matmul that reads it).
        emit_qk_tile(ps1p, wsbs["wq"], qTr, 0, 0)
        emit_qk_tile(ps1p, wsbs["wk"], kTr, 0, 0)
        emit_qk_tile(ps1p, wsbs["wk"], kTr, 0, 1)
        def emit_scores(hp, c, n, exp_tiles):
            et = expp.tile([128, 16, 512], BF16, tag="expT", name=f"expT_{r}")
            exp_tiles[n] = et
            for start, size in SC_BATCHES:
                sc = scp.tile([128, size, 512], F32, tag="sc", name=f"sc_{r}")
                for k in range(size):
                    slot = start + k
                    t2_, h = slot // 2, slot % 2
                    tg = c * TPC + t2_
                    nc.tensor.matmul(
                        sc[:, k, :],
                        lhsT=kTr[64 * h:64 * (h + 1), hp, tg * 128:(tg + 1) * 128],
                        rhs=qTr[64 * h:64 * (h + 1), hp, n * 512:(n + 1) * 512],
                        start=True, stop=True,
                    )
                nc.scalar.activation(
                    out=et[:, start:start + size, :],
                    in_=sc[:, 0:size, :],
                    func=AF.Exp,
                )

        def emit_wv(hp, c, n, exp_tiles):
            et = exp_tiles.pop(n)
            for m2 in range(4):
                m = n * 4 + m2
                # both heads packed in one PSUM bank: head h at cols [66h, 66h+65)
                psB = ps1p.tile([128, 132], F32, tag="ps1", name=f"psB_{r}")
                psBr = psB[:].rearrange("p (h e) -> p h e", e=66)
                for h in range(2):
                    hl = hp * 2 + h
                    for t2_ in range(TPC):
                        tg = c * TPC + t2_
                        nc.tensor.matmul(
                            psB[:, h * 66:h * 66 + 65],
                            lhsT=et[:, 2 * t2_ + h, m2 * 128:(m2 + 1) * 128],
                            rhs=vON[:, tg, hl * 65:(hl + 1) * 65],
                            start=(h == 0 and t2_ == 0),
                            stop=(h == 1 and t2_ == TPC - 1),
                            skip_group_check=True,
                        )
                rec = normp.tile([128, 2], F32, tag="rec", name=f"rec_{r}")
                nc.vector.reciprocal(rec[:, :], psBr[:, :, 64:65])
                for h in range(2):
                    hl = hp * 2 + h
                    dstp = attn[:, m, hl * 64:(hl + 1) * 64]
                    if c == 0:
                        nc.vector.tensor_scalar_mul(dstp, psBr[:, h, 0:64], rec[:, h:h + 1])
                    else:
                        nc.vector.scalar_tensor_tensor(
                            out=dstp, in0=psBr[:, h, 0:64], scalar=rec[:, h:h + 1],
                            in1=dstp, op0=OP.mult, op1=OP.add,
                        )

        def emit_out_quarter(n):
            """Transpose + output projection for sq tiles m in quarter n."""
            for m in range(n * 4, n * 4 + 4):
                for jt in range(2):
                    pstile = ps1p.tile([128, 128], BF16, tag="ps1", name=f"pt_{r}")
                    nc.tensor.transpose(pstile[:], attn[:, m, jt * 128:(jt + 1) * 128], ident[:])
                    nc.vector.tensor_copy(attnT[:, jt, m * 128:(m + 1) * 128], pstile[:])
            for m in range(n * 4, n * 4 + 4):
                for nn in range(2):
                    pso = ps1p.tile([128, 512], F32, tag="ps1", name=f"po_{r}")
                    for jt in range(2):
                        nc.tensor.matmul(
                            pso[:],
                            lhsT=attnT[:, jt, m * 128:(m + 1) * 128],
                            rhs=wo_sb[:, jt, nn * 512:(nn + 1) * 512],
                            start=(jt == 0),
                            stop=(jt == 1),
                        )
                    osb = osbp.tile([128, 512], BF16, tag="osb", name=f"osb_{r}")
                    if (m + nn) % 2 == 0:
                        nc.scalar.copy(osb[:], pso[:])
                    else:
                        nc.vector.tensor_copy(osb[:], pso[:])
                    nc.sync.dma_start(out_d[m * 128:(m + 1) * 128, nn * 512:(nn + 1) * 512], osb[:])

        # per-(hp, c) list of "extra" work items interleaved between exp
        # batches: v-projection tiles, deferred jt1 projections, fused
        # output-projection quarters. Slot positions: after wv(0..3).
        def qk(w, dst, jt, st):
            return lambda: emit_qk_tile(ps1p, wsbs[w], dst, jt, st)

        def vt(st):
            return lambda: emit_v_tile(ps1p, st)

        # extras[(hp, c)] = 8 slots of deferred work interleaved at:
        # [before s1, before s2-pre, after wv0, before s3, after wv1,
        #  after s3-emit/wv2, after wv3-a, after wv3-b]
        extras = {
            (0, 0): [
                [qk("wq", qTr, 0, 1)],                        # before scores n=1
                [qk("wq", qTr, 0, 2)],                        # after wv0
                [qk("wq", qTr, 0, 3), qk("wk", kTr, 0, 2)],   # after wv1
                [qk("wk", kTr, 0, 3), qk("wq", qTr, 1, 0)],   # after wv2
                [qk("wq", qTr, 1, 1)],                        # after wv3
            ],
            (0, 1): [
                [qk("wq", qTr, 1, 2)],
                [qk("wq", qTr, 1, 3)],
                [qk("wk", kTr, 1, 0), qk("wk", kTr, 1, 1)],
                [qk("wk", kTr, 1, 2)],
                [qk("wk", kTr, 1, 3)],
            ],
            (1, 0): [[], [], [], [], []],
            (1, 1): [[], [], [], [], []],
        }

        for hp in range(2):
            for c in range(2):
                last = (hp == 1 and c == 1)
                ext = extras[(hp, c)]
                ex = {}
                # v tiles needed by this (hp, c)'s wv: chunk tiles c*8..c*8+8
                pre = [vt(c * TPC + i) for i in range(TPC)] if hp == 0 else []
                emit_scores(hp, c, 0, ex)
                for f in ext[0]:
                    f()
                emit_scores(hp, c, 1, ex)
                for f in pre:
                    f()
                emit_wv(hp, c, 0, ex)
                for f in ext[1]:
                    f()
                if last:
                    emit_out_quarter(0)
                emit_scores(hp, c, 2, ex)
                emit_wv(hp, c, 1, ex)
                for f in ext[2]:
                    f()
                if last:
                    emit_out_quarter(1)
                emit_scores(hp, c, 3, ex)
                emit_wv(hp, c, 2, ex)
                for f in ext[3]:
                    f()
                if last:
                    emit_out_quarter(2)
                emit_wv(hp, c, 3, ex)
                for f in ext[4]:
                    f()
                if last:
                    emit_out_quarter(3)


def _build_nc(reps=1):
    nc = bacc.Bacc("TRN2", target_bir_lowering=False, debug=False, num_devices=8)

    aps = (
        nc.dram_tensor("xT", [D, S], BF16, kind="ExternalInput").ap(),
        nc.dram_tensor("wq", [D, JL], BF16, kind="ExternalInput").ap(),
        nc.dram_tensor("wk", [D, JL], BF16, kind="ExternalInput").ap(),
        nc.dram_tensor("wv", [D, JL], BF16, kind="ExternalInput").ap(),
        nc.dram_tensor("wo", [JL, D], BF16, kind="ExternalInput").ap(),
        nc.dram_tensor("c2", [128, S], BF16, kind="ExternalInput").ap(),
        nc.dram_tensor("s2", [128, S], BF16, kind="ExternalInput").ap(),
        nc.dram_tensor("out", [S, D], BF16, kind="ExternalOutput").ap(),
    )

    with (
        tile.TileContext(nc) as tc,
        tc.tile_pool(name="persist", bufs=1) as persist,
        tc.tile_pool(name="rope", bufs=3) as rope,
    ):
        for rep in range(reps):
            _emit_body(nc, tc, persist, rope, aps, rep)

    nc.compile()
    return nc


def _get_nc(reps=1):
    if reps not in _CACHED:
        _CACHED[reps] = _build_nc(reps)
    return _CACHED[reps]


def _host_prep(hidden_states, freqs_cis, Wq, Wk, Wv, Wo):
    bf16 = ml_dtypes.bfloat16
    hs = np.asarray(hidden_states, dtype=np.float32)
    fc = np.asarray(freqs_cis, dtype=np.float32)
    Wq = np.asarray(Wq, dtype=np.float32)
    Wk = np.asarray(Wk, dtype=np.float32)
    Wv = np.asarray(Wv, dtype=np.float32)
    Wo = np.asarray(Wo, dtype=np.float32)

    cos, sin = fc[:, :, 0], fc[:, :, 1]                      # [S, 32]
    i_idx = np.arange(128) % 32
    sign = np.where((np.arange(128) % 64) < 32, -1.0, 1.0).astype(np.float32)
    c2 = np.ascontiguousarray(cos.T[i_idx]).astype(bf16)     # [128, S]
    s2 = np.ascontiguousarray(sin.T[i_idx] * sign[:, None]).astype(bf16)

    xTs = [np.ascontiguousarray(hs[b].T).astype(bf16) for b in range(B)]

    in_maps = []
    for core in range(8):
        b, g = core // 4, core % 4
        perm = []
        for h in range(4 * g, 4 * g + 4):
            perm += [h * 64 + 2 * i for i in range(32)]
            perm += [h * 64 + 2 * i + 1 for i in range(32)]
        perm = np.array(perm)
        jcols = slice(g * JL, (g + 1) * JL)
        in_maps.append({
            "xT": xTs[b],
            "wq": np.ascontiguousarray(Wq[:, perm] * (HD ** -0.5)).astype(bf16),
            "wk": np.ascontiguousarray(Wk[:, perm]).astype(bf16),
            "wv": np.ascontiguousarray(Wv[:, jcols]).astype(bf16),
            "wo": np.ascontiguousarray(Wo[jcols, :]).astype(bf16),
            "c2": c2,
            "s2": s2,
        })
    return in_maps


def kernel(hidden_states, freqs_cis, Wq, Wk, Wv, Wo, _trace=False, _reps=1):
    nc = _get_nc(_reps)
    in_maps = _host_prep(hidden_states, freqs_cis, Wq, Wk, Wv, Wo)
    if _trace:
        try:
            from antenv.axon_hooks import get_axon_ntff_profile_hook  # noqa: F401
        except ImportError:
            _trace = False
    res = run_bass_kernel_spmd(nc, in_maps, core_ids=list(range(8)), trace=_trace)
    outs = [r["out"].astype(np.float32) for r in res.results]
    full = np.zeros((B, S, D), dtype=np.float32)
    for core in range(8):
        full[core // 4] += outs[core]
    if _trace:
        kernel._last_results = res
    return full


# revision 24
# speedup vs baseline: 1566.3308x; 1.0337x over previous
"""Trainium2 Bass kernel for chunked flash-attention block (B=2, S=2048, D=1024, H=16).

Sharding: 8 cores = 2 batches x 4 head-groups (4 heads each). Each core computes
its heads' QKV projections + RoPE + per-chunk-softmax attention + its slice of the
output projection; the host sums the 4 partial out-projections per batch.

All device-side activations are kept transposed ([feature, seq]) so every matmul
contraction lands on the partition dimension with no on-device transposes of the
inputs. Emission order streams the attention (exp on ScalarE is the bottleneck):
head-pair 0 attention starts right after its projections; head-pair 1 projections
are interleaved into head-pair 0's attention; the output projection is fused into
the last attention pass.
"""

import numpy as np
import ml_dtypes

import concourse.bass as bass
import concourse.tile as tile
from concourse import bacc, mybir
from concourse.bass_utils import run_bass_kernel_spmd
from concourse.masks import make_identity

dt = mybir.dt
F32 = dt.float32
BF16 = dt.bfloat16
AF = mybir.ActivationFunctionType
OP = mybir.AluOpType

B, S, D, H, HD = 2, 2048, 1024, 16, 64
CHUNK = 1024
NHL = 4              # local heads per core
JL = NHL * HD        # 256 local projected dims
ND = D // 128        # 8 k-tiles for the projections
NSQ = S // 512       # 4 sq n-tiles
NSK = S // 128       # 16 sk p-tiles
NCH = S // CHUNK     # 2 key chunks
TPC = CHUNK // 128   # 8 sk tiles per chunk
NM = S // 128        # 16 sq p-tiles

# exp batches per (hp, chunk, n): 16 (t, h) score slots -> 6 ACT instructions
SC_BATCHES = ((0, 2), (2, 3), (5, 3), (8, 3), (11, 3), (14, 2))

_CACHED = {}


def _emit_body(nc, tc, persist, rope, aps, rep):
    """Emit one full iteration of the kernel into the open TileContext."""
    xT_d, wq_d, wk_d, wv_d, wo_d, c2_d, s2_d, out_d = aps
    r = f"r{rep}"

    # DMA order matters: xT + wq gate the first projection matmuls.
    xsb = persist.tile([128, ND, S], BF16, tag="xT", name=f"xT_{r}")
    xT_r = xT_d.rearrange("(t p) s -> p t s", p=128)
    for di in range(ND):
        nc.sync.dma_start(xsb[:, di, :], xT_r[:, di, :])
    wsbs = {}
    for nm, d_ap in (("wq", wq_d), ("wk", wk_d), ("wv", wv_d)):
        wsbs[nm] = persist.tile([128, ND, JL], BF16, tag=nm, name=f"{nm}_{r}")
        nc.sync.dma_start(wsbs[nm][:], d_ap.rearrange("(t p) j -> p t j", p=128))
    c2 = persist.tile([128, S], BF16, tag="c2", name=f"c2_{r}")
    s2 = persist.tile([128, S], BF16, tag="s2", name=f"s2_{r}")
    nc.sync.dma_start(c2[:], c2_d)
    nc.sync.dma_start(s2[:], s2_d)
    wo_sb = persist.tile([128, 2, D], BF16, tag="wo", name=f"wo_{r}")
    nc.sync.dma_start(wo_sb[:], wo_d.rearrange("(t p) n -> p t n", p=128))
    ident = persist.tile([128, 128], BF16, tag="ident", name=f"ident_{r}")
    make_identity(nc, ident[:])

    qTr = persist.tile([128, 2, S], BF16, tag="qTr", name=f"qTr_{r}")
    kTr = persist.tile([128, 2, S], BF16, tag="kTr", name=f"kTr_{r}")
    vON = persist.tile([128, NSK, NHL * 65], BF16, tag="vON", name=f"vON_{r}")
    attn = persist.tile([128, NM, JL], BF16, tag="attn", name=f"attn_{r}")
    attnT = persist.tile([128, 2, S], BF16, tag="attnT", name=f"attnT_{r}")

    vON_r = vON[:].rearrange("p t (h c) -> p (t h) c", c=65)
    nc.vector.memset(vON_r[:, :, 64:65], 1.0)

    def emit_qk_tile(pool, wsb, dst, jt, st):
        """One [128, 512] q/k projection tile + RoPE into dst (qTr/kTr)."""
        sl = slice(st * 512, (st + 1) * 512)
        ps = pool.tile([128, 512], F32, tag="ps1", name=f"pqk_{r}")
        for di in range(ND):
            nc.tensor.matmul(
                ps[:],
                lhsT=wsb[:, di, jt * 128:(jt + 1) * 128],
                rhs=xsb[:, di, sl],
                start=(di == 0),
                stop=(di == ND - 1),
            )
        # RoPE in bf16 (fast DVE mode). rot[p] = q[p]*cos - (q*s2)[swap(p)]
        # where swap flips the a/b 32-row halves within each head; the
        # partition swap rides on GpSimd (single-input shifted copies).
        qb = rope.tile([128, 512], BF16, tag="qb", name=f"qb_{r}")
        nc.vector.tensor_copy(qb[:], ps[:])
        w2 = rope.tile([128, 512], BF16, tag="w2", name=f"w2_{r}")
        nc.vector.tensor_mul(w2[:], qb[:], s2[:, sl])
        u = rope.tile([128, 512], BF16, tag="u", name=f"u_{r}")
        for blk in range(4):
            o = blk * 32
            so = o ^ 32
            eng = nc.gpsimd if blk < 2 else nc.vector
            eng.tensor_copy(u[o:o + 32, :], w2[so:so + 32, :])
        t2 = rope.tile([128, 512], BF16, tag="t2", name=f"t2_{r}")
        nc.vector.tensor_mul(t2[:], qb[:], c2[:, sl])
        nc.vector.tensor_sub(dst[:, jt, sl], t2[:], u[:])

    def emit_v_tile(pool, st):
        psv = pool.tile([128, JL], F32, tag="ps1", name=f"pv_{r}")
        for di in range(ND):
            nc.tensor.matmul(
                psv[:],
                lhsT=xsb[:, di, st * 128:(st + 1) * 128],
                rhs=wsbs["wv"][:, di, :],
                start=(di == 0),
                stop=(di == ND - 1),
            )
        nc.vector.tensor_copy(
            vON_r[:, st * NHL:(st + 1) * NHL, 0:64],
            psv[:].rearrange("p (h e) -> p h e", e=64),
        )

    # ---------------- attention (projections stream through the same pool) --
    with (
        tc.tile_pool(name=f"sc_{r}", bufs=2, space="PSUM") as scp,
        tc.tile_pool(name=f"ps1_{r}", bufs=2, space="PSUM") as ps1p,
        tc.tile_pool(name=f"expp_{r}", bufs=3) as expp,
        tc.tile_pool(name=f"normp_{r}", bufs=8) as normp,
        tc.tile_pool(name=f"osb_{r}", bufs=8) as osbp,
    ):
        # phase 1: only the projections gating the very first score batch —
        # PE executes in emission order, so everything else streams between
        # score/exp groups (each projection must still be emitted before the
        # first matmul that reads it).
        emit_qk_tile(ps1p, wsbs["wq"], qTr, 0, 0)
        emit_qk_tile(ps1p, wsbs["wk"], kTr, 0, 0)
        emit_qk_tile(ps1p, wsbs["wk"], kTr, 0, 1)
        def emit_scores(hp, c, n, exp_tiles):
            et = expp.tile([128, 16, 512], BF16, tag="expT", name=f"expT_{r}")
            exp_tiles[n] = et
            for start, size in SC_BATCHES:
                sc = scp.tile([128, size, 512], F32, tag="sc", name=f"sc_{r}")
                for k in range(size):
                    slot = start + k
                    t2_, h = slot // 2, slot % 2
                    tg = c * TPC + t2_
                    nc.tensor.matmul(
                        sc[:, k, :],
                        lhsT=kTr[64 * h:64 * (h + 1), hp, tg * 128:(tg + 1) * 128],
                        rhs=qTr[64 * h:64 * (h + 1), hp, n * 512:(n + 1) * 512],
                        start=True, stop=True,
                    )
                nc.scalar.activation(
                    out=et[:, start:start + size, :],
                    in_=sc[:, 0:size, :],
                    func=AF.Exp,
                )

        def emit_wv(hp, c, n, exp_tiles):
            et = exp_tiles.pop(n)
            for m2 in range(4):
                m = n * 4 + m2
                # both heads packed in one PSUM bank: head h at cols [66h, 66h+65)
                psB = ps1p.tile([128, 132], F32, tag="ps1", name=f"psB_{r}")
                psBr = psB[:].rearrange("p (h e) -> p h e", e=66)
                for h in range(2):
                    hl = hp * 2 + h
                    for t2_ in range(TPC):
                        tg = c * TPC + t2_
                        nc.tensor.matmul(
                            psB[:, h * 66:h * 66 + 65],
                            lhsT=et[:, 2 * t2_ + h, m2 * 128:(m2 + 1) * 128],
                            rhs=vON[:, tg, hl * 65:(hl + 1) * 65],
                            start=(h == 0 and t2_ == 0),
                            stop=(h == 1 and t2_ == TPC - 1),
                            skip_group_check=True,
                        )
                rec = normp.tile([128, 2], F32, tag="rec", name=f"rec_{r}")
                nc.vector.reciprocal(rec[:, :], psBr[:, :, 64:65])
                for h in range(2):
                    hl = hp * 2 + h
                    dstp = attn[:, m, hl * 64:(hl + 1) * 64]
                    if c == 0:
                        nc.vector.tensor_scalar_mul(dstp, psBr[:, h, 0:64], rec[:, h:h + 1])
                    else:
                        nc.vector.scalar_tensor_tensor(
                            out=dstp, in0=psBr[:, h, 0:64], scalar=rec[:, h:h + 1],
                            in1=dstp, op0=OP.mult, op1=OP.add,
                        )

        def emit_out_quarter(n):
            """Transpose + output projection for sq tiles m in quarter n."""
            for m in range(n * 4, n * 4 + 4):
                for jt in range(2):
                    pstile = ps1p.tile([128, 128], BF16, tag="ps1", name=f"pt_{r}")
                    nc.tensor.transpose(pstile[:], attn[:, m, jt * 128:(jt + 1) * 128], ident[:])
                    nc.vector.tensor_copy(attnT[:, jt, m * 128:(m + 1) * 128], pstile[:])
            for m in range(n * 4, n * 4 + 4):
                for nn in range(2):
                    pso = ps1p.tile([128, 512], F32, tag="ps1", name=f"po_{r}")
                    for jt in range(2):
                        nc.tensor.matmul(
                            pso[:],
                            lhsT=attnT[:, jt, m * 128:(m + 1) * 128],
                            rhs=wo_sb[:, jt, nn * 512:(nn + 1) * 512],
                            start=(jt == 0),
                            stop=(jt == 1),
                        )
                    osb = osbp.tile([128, 512], BF16, tag="osb", name=f"osb_{r}")
                    if n >= 2 and (m + nn) % 2 == 0:
                        nc.scalar.copy(osb[:], pso[:])
                    else:
                        nc.vector.tensor_copy(osb[:], pso[:])
                    nc.sync.dma_start(out_d[m * 128:(m + 1) * 128, nn * 512:(nn + 1) * 512], osb[:])

        # per-(hp, c) list of "extra" work items interleaved between exp
        # batches: v-projection tiles, deferred jt1 projections, fused
        # output-projection quarters. Slot positions: after wv(0..3).
        def qk(w, dst, jt, st):
            return lambda: emit_qk_tile(ps1p, wsbs[w], dst, jt, st)

        def vt(st):
            return lambda: emit_v_tile(ps1p, st)

        # extras[(hp, c)] = 8 slots of deferred work interleaved at:
        # [before s1, before s2-pre, after wv0, before s3, after wv1,
        #  after s3-emit/wv2, after wv3-a, after wv3-b]
        extras = {
            (0, 0): [
                [qk("wq", qTr, 0, 1)],                        # before scores n=1
                [qk("wq", qTr, 0, 2)],                        # after wv0
                [qk("wq", qTr, 0, 3), qk("wk", kTr, 0, 2)],   # after wv1
                [qk("wk", kTr, 0, 3), qk("wq", qTr, 1, 0)],   # after wv2
                [qk("wq", qTr, 1, 1)],                        # after wv3
            ],
            (0, 1): [
                [qk("wq", qTr, 1, 2)],
                [qk("wq", qTr, 1, 3)],
                [qk("wk", kTr, 1, 0), qk("wk", kTr, 1, 1)],
                [qk("wk", kTr, 1, 2)],
                [qk("wk", kTr, 1, 3)],
            ],
            (1, 0): [[], [], [], [], []],
            (1, 1): [[], [], [], [], []],
        }

        for hp in range(2):
            for c in range(2):
                last = (hp == 1 and c == 1)
                ext = extras[(hp, c)]
                ex = {}
                # v tiles needed by this (hp, c)'s wv: chunk tiles c*8..c*8+8
                pre = [vt(c * TPC + i) for i in range(TPC)] if hp == 0 else []
                emit_scores(hp, c, 0, ex)
                for f in ext[0]:
                    f()
                emit_scores(hp, c, 1, ex)
                for f in pre:
                    f()
                emit_wv(hp, c, 0, ex)
                for f in ext[1]:
                    f()
                if last:
                    emit_out_quarter(0)
                emit_scores(hp, c, 2, ex)
                emit_wv(hp, c, 1, ex)
                for f in ext[2]:
                    f()
                if last:
                    emit_out_quarter(1)
                emit_scores(hp, c, 3, ex)
                emit_wv(hp, c, 2, ex)
                for f in ext[3]:
                    f()
                if last:
                    emit_out_quarter(2)
                emit_wv(hp, c, 3, ex)
                for f in ext[4]:
                    f()
                if last:
                    emit_out_quarter(3)


def _build_nc(reps=1):
    nc = bacc.Bacc("TRN2", target_bir_lowering=False, debug=False, num_devices=8)

    aps = (
        nc.dram_tensor("xT", [D, S], BF16, kind="ExternalInput").ap(),
        nc.dram_tensor("wq", [D, JL], BF16, kind="ExternalInput").ap(),
        nc.dram_tensor("wk", [D, JL], BF16, kind="ExternalInput").ap(),
        nc.dram_tensor("wv", [D, JL], BF16, kind="ExternalInput").ap(),
        nc.dram_tensor("wo", [JL, D], BF16, kind="ExternalInput").ap(),
        nc.dram_tensor("c2", [128, S], BF16, kind="ExternalInput").ap(),
        nc.dram_tensor("s2", [128, S], BF16, kind="ExternalInput").ap(),
        nc.dram_tensor("out", [S, D], BF16, kind="ExternalOutput").ap(),
    )

    with (
        tile.TileContext(nc) as tc,
        tc.tile_pool(name="persist", bufs=1) as persist,
        tc.tile_pool(name="rope", bufs=3) as rope,
    ):
        for rep in range(reps):
            _emit_body(nc, tc, persist, rope, aps, rep)

    nc.compile()
    return nc


def _get_nc(reps=1):
    if reps not in _CACHED:
        _CACHED[reps] = _build_nc(reps)
    return _CACHED[reps]


def _host_prep(hidden_states, freqs_cis, Wq, Wk, Wv, Wo):
    bf16 = ml_dtypes.bfloat16
    hs = np.asarray(hidden_states, dtype=np.float32)
    fc = np.asarray(freqs_cis, dtype=np.float32)
    Wq = np.asarray(Wq, dtype=np.float32)
    Wk = np.asarray(Wk, dtype=np.float32)
    Wv = np.asarray(Wv, dtype=np.float32)
    Wo = np.asarray(Wo, dtype=np.float32)

    cos, sin = fc[:, :, 0], fc[:, :, 1]                      # [S, 32]
    i_idx = np.arange(128) % 32
    sign = np.where((np.arange(128) % 64) < 32, -1.0, 1.0).astype(np.float32)
    c2 = np.ascontiguousarray(cos.T[i_idx]).astype(bf16)     # [128, S]
    s2 = np.ascontiguousarray(sin.T[i_idx] * sign[:, None]).astype(bf16)

    xTs = [np.ascontiguousarray(hs[b].T).astype(bf16) for b in range(B)]

    in_maps = []
    for core in range(8):
        b, g = core // 4, core % 4
        perm = []
        for h in range(4 * g, 4 * g + 4):
            perm += [h * 64 + 2 * i for i in range(32)]
            perm += [h * 64 + 2 * i + 1 for i in range(32)]
        perm = np.array(perm)
        jcols = slice(g * JL, (g + 1) * JL)
        in_maps.append({
            "xT": xTs[b],
            "wq": np.ascontiguousarray(Wq[:, perm] * (HD ** -0.5)).astype(bf16),
            "wk": np.ascontiguousarray(Wk[:, perm]).astype(bf16),
            "wv": np.ascontiguousarray(Wv[:, jcols]).astype(bf16),
            "wo": np.ascontiguousarray(Wo[jcols, :]).astype(bf16),
            "c2": c2,
            "s2": s2,
        })
    return in_maps


def kernel(hidden_states, freqs_cis, Wq, Wk, Wv, Wo, _trace=False, _reps=1):
    nc = _get_nc(_reps)
    in_maps = _host_prep(hidden_states, freqs_cis, Wq, Wk, Wv, Wo)
    if _trace:
        try:
            from antenv.axon_hooks import get_axon_ntff_profile_hook  # noqa: F401
        except ImportError:
            _trace = False
    res = run_bass_kernel_spmd(nc, in_maps, core_ids=list(range(8)), trace=_trace)
    outs = [r["out"].astype(np.float32) for r in res.results]
    full = np.zeros((B, S, D), dtype=np.float32)
    for core in range(8):
        full[core // 4] += outs[core]
    if _trace:
        kernel._last_results = res
    return full


# revision 29
# speedup vs baseline: 1612.3133x; 1.0294x over previous
"""Trainium2 Bass kernel for chunked flash-attention block (B=2, S=2048, D=1024, H=16).

Sharding: 8 cores = 2 batches x 4 head-groups (4 heads each). Each core computes
its heads' QKV projections + RoPE + per-chunk-softmax attention + its slice of the
output projection; the host sums the 4 partial out-projections per batch.

All device-side activations are kept transposed ([feature, seq]) so every matmul
contraction lands on the partition dimension with no on-device transposes of the
inputs. Emission order streams the attention (exp on ScalarE is the bottleneck):
head-pair 0 attention starts right after its projections; head-pair 1 projections
are interleaved into head-pair 0's attention; the output projection is fused into
the last attention pass.
"""

import numpy as np
import ml_dtypes

import concourse.bass as bass
import concourse.tile as tile
from concourse import bacc, mybir
from concourse.bass_utils import run_bass_kernel_spmd
from concourse.masks import make_identity

dt = mybir.dt
F32 = dt.float32
BF16 = dt.bfloat16
AF = mybir.ActivationFunctionType
OP = mybir.AluOpType

B, S, D, H, HD = 2, 2048, 1024, 16, 64
CHUNK = 1024
NHL = 4              # local heads per core
JL = NHL * HD        # 256 local projected dims
ND = D // 128        # 8 k-tiles for the projections
NSQ = S // 512       # 4 sq n-tiles
NSK = S // 128       # 16 sk p-tiles
NCH = S // CHUNK     # 2 key chunks
TPC = CHUNK // 128   # 8 sk tiles per chunk
NM = S // 128        # 16 sq p-tiles

# exp batches per (hp, chunk, n): 16 (t, h) score slots -> 6 ACT instructions
SC_BATCHES = ((0, 2), (2, 3), (5, 3), (8, 3), (11, 3), (14, 2))

_CACHED = {}


def _emit_body(nc, tc, persist, rope, aps, rep):
    """Emit one full iteration of the kernel into the open TileContext."""
    xT_d, wq_d, wk_d, wv_d, wo_d, c2_d, s2_d, out_d = aps
    r = f"r{rep}"

    # DMA order matters: wq lands first (it feeds the PE warm-up matmuls),
    # then xT which gates the first projection matmuls.
    wsbs = {}
    wsbs["wq"] = persist.tile([128, ND, JL], BF16, tag="wq", name=f"wq_{r}")
    nc.sync.dma_start(wsbs["wq"][:], wq_d.rearrange("(t p) j -> p t j", p=128))
    xsb = persist.tile([128, ND, S], BF16, tag="xT", name=f"xT_{r}")
    xT_r = xT_d.rearrange("(t p) s -> p t s", p=128)
    for di in range(ND):
        nc.sync.dma_start(xsb[:, di, :], xT_r[:, di, :])
    for nm, d_ap in (("wk", wk_d), ("wv", wv_d)):
        wsbs[nm] = persist.tile([128, ND, JL], BF16, tag=nm, name=f"{nm}_{r}")
        nc.sync.dma_start(wsbs[nm][:], d_ap.rearrange("(t p) j -> p t j", p=128))
    c2 = persist.tile([128, S], BF16, tag="c2", name=f"c2_{r}")
    s2 = persist.tile([128, S], BF16, tag="s2", name=f"s2_{r}")
    nc.sync.dma_start(c2[:], c2_d)
    nc.sync.dma_start(s2[:], s2_d)
    wo_sb = persist.tile([128, 2, D], BF16, tag="wo", name=f"wo_{r}")
    nc.sync.dma_start(wo_sb[:], wo_d.rearrange("(t p) n -> p t n", p=128))
    ident = persist.tile([128, 128], BF16, tag="ident", name=f"ident_{r}")
    make_identity(nc, ident[:])

    qTr = persist.tile([128, 2, S], BF16, tag="qTr", name=f"qTr_{r}")
    kTr = persist.tile([128, 2, S], BF16, tag="kTr", name=f"kTr_{r}")
    vON = persist.tile([128, NSK, NHL * 65], BF16, tag="vON", name=f"vON_{r}")
    attn = persist.tile([128, NM, JL], BF16, tag="attn", name=f"attn_{r}")
    attnT = persist.tile([128, 2, S], BF16, tag="attnT", name=f"attnT_{r}")

    vON_r = vON[:].rearrange("p t (h c) -> p (t h) c", c=65)
    nc.vector.memset(vON_r[:, :, 64:65], 1.0)

    def emit_qk_tile(pool, wsb, dst, jt, st, fast_rope=False):
        """One [128, 512] q/k projection tile + RoPE into dst (qTr/kTr)."""
        sl = slice(st * 512, (st + 1) * 512)
        ps = pool.tile([128, 512], F32, tag="ps1", name=f"pqk_{r}")
        for di in range(ND):
            nc.tensor.matmul(
                ps[:],
                lhsT=wsb[:, di, jt * 128:(jt + 1) * 128],
                rhs=xsb[:, di, sl],
                start=(di == 0),
                stop=(di == ND - 1),
            )
        # RoPE in bf16 (fast DVE mode). rot[p] = q[p]*cos - (q*s2)[swap(p)]
        # where swap flips the a/b 32-row halves within each head; the
        # partition swap rides on GpSimd (single-input shifted copies).
        qb = rope.tile([128, 512], BF16, tag="qb", name=f"qb_{r}")
        nc.vector.tensor_copy(qb[:], ps[:])
        w2 = rope.tile([128, 512], BF16, tag="w2", name=f"w2_{r}")
        nc.vector.tensor_mul(w2[:], qb[:], s2[:, sl])
        u = rope.tile([128, 512], BF16, tag="u", name=f"u_{r}")
        for blk in range(4):
            o = blk * 32
            so = o ^ 32
            eng = nc.vector if (fast_rope or blk >= 2) else nc.gpsimd
            eng.tensor_copy(u[o:o + 32, :], w2[so:so + 32, :])
        t2 = rope.tile([128, 512], BF16, tag="t2", name=f"t2_{r}")
        nc.vector.tensor_mul(t2[:], qb[:], c2[:, sl])
        nc.vector.tensor_sub(dst[:, jt, sl], t2[:], u[:])

    def emit_v_tile(pool, st):
        psv = pool.tile([128, JL], F32, tag="ps1", name=f"pv_{r}")
        for di in range(ND):
            nc.tensor.matmul(
                psv[:],
                lhsT=xsb[:, di, st * 128:(st + 1) * 128],
                rhs=wsbs["wv"][:, di, :],
                start=(di == 0),
                stop=(di == ND - 1),
            )
        nc.vector.tensor_copy(
            vON_r[:, st * NHL:(st + 1) * NHL, 0:64],
            psv[:].rearrange("p (h e) -> p h e", e=64),
        )

    # ---------------- attention (projections stream through the same pool) --
    with (
        tc.tile_pool(name=f"sc_{r}", bufs=2, space="PSUM") as scp,
        tc.tile_pool(name=f"ps1_{r}", bufs=2, space="PSUM") as ps1p,
        tc.tile_pool(name=f"expp_{r}", bufs=3) as expp,
        tc.tile_pool(name=f"normp_{r}", bufs=8) as normp,
        tc.tile_pool(name=f"osb_{r}", bufs=8) as osbp,
    ):
        # PE warm-up: the HAM clock gate keeps a cold PE at half rate for the
        # first ~3.4us of activity. The xT load takes ~13us during which PE
        # would idle cold; run throwaway matmuls on the (early-arriving) wq
        # tile so the real projections start at full clock.
        warm = scp.tile([128, 3, 512], F32, tag="sc", name=f"warm_{r}")
        for i in range(56):
            nc.tensor.matmul(
                warm[:, i % 3, 0:256],
                lhsT=wsbs["wq"][:, 0, 0:128],
                rhs=wsbs["wq"][:, 0, 0:256],
                start=True, stop=True,
            )

        # phase 1: only the projections gating the very first score batch —
        # PE executes in emission order, so everything else streams between
        # score/exp groups (each projection must still be emitted before the
        # first matmul that reads it).
        emit_qk_tile(ps1p, wsbs["wq"], qTr, 0, 0, fast_rope=True)
        emit_qk_tile(ps1p, wsbs["wk"], kTr, 0, 0, fast_rope=True)
        emit_qk_tile(ps1p, wsbs["wk"], kTr, 0, 1, fast_rope=True)
        def emit_scores(hp, c, n, exp_tiles):
            et = expp.tile([128, 16, 512], BF16, tag="expT", name=f"expT_{r}")
            exp_tiles[n] = et
            for start, size in SC_BATCHES:
                sc = scp.tile([128, size, 512], F32, tag="sc", name=f"sc_{r}")
                for k in range(size):
                    slot = start + k
                    t2_, h = slot // 2, slot % 2
                    tg = c * TPC + t2_
                    nc.tensor.matmul(
                        sc[:, k, :],
                        lhsT=kTr[64 * h:64 * (h + 1), hp, tg * 128:(tg + 1) * 128],
                        rhs=qTr[64 * h:64 * (h + 1), hp, n * 512:(n + 1) * 512],
                        start=True, stop=True,
                    )
                nc.scalar.activation(
                    out=et[:, start:start + size, :],
                    in_=sc[:, 0:size, :],
                    func=AF.Exp,
                )

        def emit_wv(hp, c, n, exp_tiles):
            et = exp_tiles.pop(n)
            for m2 in range(4):
                m = n * 4 + m2
                # both heads packed in one PSUM bank: head h at cols [66h, 66h+65)
                psB = ps1p.tile([128, 132], F32, tag="ps1", name=f"psB_{r}")
                psBr = psB[:].rearrange("p (h e) -> p h e", e=66)
                for h in range(2):
                    hl = hp * 2 + h
                    for t2_ in range(TPC):
                        tg = c * TPC + t2_
                        nc.tensor.matmul(
                            psB[:, h * 66:h * 66 + 65],
                            lhsT=et[:, 2 * t2_ + h, m2 * 128:(m2 + 1) * 128],
                            rhs=vON[:, tg, hl * 65:(hl + 1) * 65],
                            start=(h == 0 and t2_ == 0),
                            stop=(h == 1 and t2_ == TPC - 1),
                            skip_group_check=True,
                        )
                rec = normp.tile([128, 2], F32, tag="rec", name=f"rec_{r}")
                nc.vector.reciprocal(rec[:, :], psBr[:, :, 64:65])
                for h in range(2):
                    hl = hp * 2 + h
                    dstp = attn[:, m, hl * 64:(hl + 1) * 64]
                    if c == 0:
                        nc.vector.tensor_scalar_mul(dstp, psBr[:, h, 0:64], rec[:, h:h + 1])
                    else:
                        nc.vector.scalar_tensor_tensor(
                            out=dstp, in0=psBr[:, h, 0:64], scalar=rec[:, h:h + 1],
                            in1=dstp, op0=OP.mult, op1=OP.add,
                        )

        def emit_out_quarter(n):
            """Transpose + output projection for sq tiles m in quarter n."""
            for m in range(n * 4, n * 4 + 4):
                for jt in range(2):
                    pstile = ps1p.tile([128, 128], BF16, tag="ps1", name=f"pt_{r}")
                    nc.tensor.transpose(pstile[:], attn[:, m, jt * 128:(jt + 1) * 128], ident[:])
                    nc.vector.tensor_copy(attnT[:, jt, m * 128:(m + 1) * 128], pstile[:])
            for m in range(n * 4, n * 4 + 4):
                for nn in range(2):
                    pso = ps1p.tile([128, 512], F32, tag="ps1", name=f"po_{r}")
                    for jt in range(2):
                        nc.tensor.matmul(
                            pso[:],
                            lhsT=attnT[:, jt, m * 128:(m + 1) * 128],
                            rhs=wo_sb[:, jt, nn * 512:(nn + 1) * 512],
                            start=(jt == 0),
                            stop=(jt == 1),
                        )
                    osb = osbp.tile([128, 512], BF16, tag="osb", name=f"osb_{r}")
                    if n >= 2 and (m + nn) % 2 == 0:
                        nc.scalar.copy(osb[:], pso[:])
                    else:
                        nc.vector.tensor_copy(osb[:], pso[:])
                    nc.sync.dma_start(out_d[m * 128:(m + 1) * 128, nn * 512:(nn + 1) * 512], osb[:])

        # per-(hp, c) list of "extra" work items interleaved between exp
        # batches: v-projection tiles, deferred jt1 projections, fused
        # output-projection quarters. Slot positions: after wv(0..3).
        def qk(w, dst, jt, st):
            return lambda: emit_qk_tile(ps1p, wsbs[w], dst, jt, st)

        def vt(st):
            return lambda: emit_v_tile(ps1p, st)

        # extras[(hp, c)] = 8 slots of deferred work interleaved at:
        # [before s1, before s2-pre, after wv0, before s3, after wv1,
        #  after s3-emit/wv2, after wv3-a, after wv3-b]
        extras = {
            (0, 0): [
                [qk("wq", qTr, 0, 1)],                        # before scores n=1
                [qk("wq", qTr, 0, 2)],                        # after wv0
                [qk("wq", qTr, 0, 3), qk("wk", kTr, 0, 2)],   # after wv1
                [qk("wk", kTr, 0, 3), qk("wq", qTr, 1, 0)],   # after wv2
                [qk("wq", qTr, 1, 1)],                        # after wv3
            ],
            (0, 1): [
                [qk("wq", qTr, 1, 2)],
                [qk("wq", qTr, 1, 3)],
                [qk("wk", kTr, 1, 0), qk("wk", kTr, 1, 1)],
                [qk("wk", kTr, 1, 2)],
                [qk("wk", kTr, 1, 3)],
            ],
            (1, 0): [[], [], [], [], []],
            (1, 1): [[], [], [], [], []],
        }

        for hp in range(2):
            for c in range(2):
                last = (hp == 1 and c == 1)
                ext = extras[(hp, c)]
                ex = {}
                # v tiles needed by this (hp, c)'s wv: chunk tiles c*8..c*8+8
                pre = [vt(c * TPC + i) for i in range(TPC)] if hp == 0 else []
                emit_scores(hp, c, 0, ex)
                for f in ext[0]:
                    f()
                emit_scores(hp, c, 1, ex)
                for f in pre:
                    f()
                emit_wv(hp, c, 0, ex)
                for f in ext[1]:
                    f()
                if last:
                    emit_out_quarter(0)
                emit_scores(hp, c, 2, ex)
                emit_wv(hp, c, 1, ex)
                for f in ext[2]:
                    f()
                if last:
                    emit_out_quarter(1)
                emit_scores(hp, c, 3, ex)
                emit_wv(hp, c, 2, ex)
                for f in ext[3]:
                    f()
                if last:
                    emit_out_quarter(2)
                emit_wv(hp, c, 3, ex)
                for f in ext[4]:
                    f()
                if last:
                    emit_out_quarter(3)


def _build_nc(reps=1):
    nc = bacc.Bacc("TRN2", target_bir_lowering=False, debug=False, num_devices=8)

    aps = (
        nc.dram_tensor("xT", [D, S], BF16, kind="ExternalInput").ap(),
        nc.dram_tensor("wq", [D, JL], BF16, kind="ExternalInput").ap(),
        nc.dram_tensor("wk", [D, JL], BF16, kind="ExternalInput").ap(),
        nc.dram_tensor("wv", [D, JL], BF16, kind="ExternalInput").ap(),
        nc.dram_tensor("wo", [JL, D], BF16, kind="ExternalInput").ap(),
        nc.dram_tensor("c2", [128, S], BF16, kind="ExternalInput").ap(),
        nc.dram_tensor("s2", [128, S], BF16, kind="ExternalInput").ap(),
        nc.dram_tensor("out", [S, D], BF16, kind="ExternalOutput").ap(),
    )

    with (
        tile.TileContext(nc) as tc,
        tc.tile_pool(name="persist", bufs=1) as persist,
        tc.tile_pool(name="rope", bufs=3) as rope,
    ):
        for rep in range(reps):
            _emit_body(nc, tc, persist, rope, aps, rep)

    nc.compile()
    return nc


def _get_nc(reps=1):
    if reps not in _CACHED:
        _CACHED[reps] = _build_nc(reps)
    return _CACHED[reps]


def _host_prep(hidden_states, freqs_cis, Wq, Wk, Wv, Wo):
    bf16 = ml_dtypes.bfloat16
    hs = np.asarray(hidden_states, dtype=np.float32)
    fc = np.asarray(freqs_cis, dtype=np.float32)
    Wq = np.asarray(Wq, dtype=np.float32)
    Wk = np.asarray(Wk, dtype=np.float32)
    Wv = np.asarray(Wv, dtype=np.float32)
    Wo = np.asarray(Wo, dtype=np.float32)

    cos, sin = fc[:, :, 0], fc[:, :, 1]                      # [S, 32]
    i_idx = np.arange(128) % 32
    sign = np.where((np.arange(128) % 64) < 32, -1.0, 1.0).astype(np.float32)
    c2 = np.ascontiguousarray(cos.T[i_idx]).astype(bf16)     # [128, S]
    s2 = np.ascontiguousarray(sin.T[i_idx] * sign[:, None]).astype(bf16)

    xTs = [np.ascontiguousarray(hs[b].T).astype(bf16) for b in range(B)]

    in_maps = []
    for core in range(8):
        b, g = core // 4, core % 4
        perm = []
        for h in range(4 * g, 4 * g + 4):
            perm += [h * 64 + 2 * i for i in range(32)]
            perm += [h * 64 + 2 * i + 1 for i in range(32)]
        perm = np.array(perm)
        jcols = slice(g * JL, (g + 1) * JL)
        in_maps.append({
            "xT": xTs[b],
            "wq": np.ascontiguousarray(Wq[:, perm] * (HD ** -0.5)).astype(bf16),
            "wk": np.ascontiguousarray(Wk[:, perm]).astype(bf16),
            "wv": np.ascontiguousarray(Wv[:, jcols]).astype(bf16),
            "wo": np.ascontiguousarray(Wo[jcols, :]).astype(bf16),
            "c2": c2,
            "s2": s2,
        })
    return in_maps


def kernel(hidden_states, freqs_cis, Wq, Wk, Wv, Wo, _trace=False, _reps=1):
    nc = _get_nc(_reps)
    in_maps = _host_prep(hidden_states, freqs_cis, Wq, Wk, Wv, Wo)
    if _trace:
        try:
            from antenv.axon_hooks import get_axon_ntff_profile_hook  # noqa: F401
        except ImportError:
            _trace = False
    res = run_bass_kernel_spmd(nc, in_maps, core_ids=list(range(8)), trace=_trace)
    outs = [r["out"].astype(np.float32) for r in res.results]
    full = np.zeros((B, S, D), dtype=np.float32)
    for core in range(8):
        full[core // 4] += outs[core]
    if _trace:
        kernel._last_results = res
    return full


# revision 38
# speedup vs baseline: 1699.1118x; 1.0538x over previous
"""Trainium2 Bass kernel for chunked flash-attention block (B=2, S=2048, D=1024, H=16).

Sharding: 8 cores = 2 batches x 4 head-groups (4 heads each). Each core computes
its heads' QKV projections + RoPE + per-chunk-softmax attention + its slice of the
output projection; the host sums the 4 partial out-projections per batch.

All device-side activations are kept transposed ([feature, seq]) so every matmul
contraction lands on the partition dimension with no on-device transposes of the
inputs. Emission order streams the attention (exp on ScalarE is the bottleneck):
head-pair 0 attention starts right after its projections; head-pair 1 projections
are interleaved into head-pair 0's attention; the output projection is fused into
the last attention pass.
"""

import numpy as np
import ml_dtypes

import concourse.bass as bass
import concourse.tile as tile
from concourse import bacc, mybir
from concourse.bass_utils import run_bass_kernel_spmd
from concourse.masks import make_identity

dt = mybir.dt
F32 = dt.float32
BF16 = dt.bfloat16
AF = mybir.ActivationFunctionType
OP = mybir.AluOpType

B, S, D, H, HD = 2, 2048, 1024, 16, 64
CHUNK = 1024
NHL = 4              # local heads per core
JL = NHL * HD        # 256 local projected dims
ND = D // 128        # 8 k-tiles for the projections
NSQ = S // 512       # 4 sq n-tiles
NSK = S // 128       # 16 sk p-tiles
NCH = S // CHUNK     # 2 key chunks
TPC = CHUNK // 128   # 8 sk tiles per chunk
NM = S // 128        # 16 sq p-tiles

# exp batches per (hp, chunk, n): 16 (t, h) score slots -> 6 ACT instructions
SC_BATCHES = ((0, 2), (2, 3), (5, 3), (8, 3), (11, 3), (14, 2))

_CACHED = {}


def _emit_body(nc, tc, persist, rope, aps, rep):
    """Emit one full iteration of the kernel into the open TileContext."""
    xT_d, wq_d, wk_d, wv_d, wo_d, c2_d, s2_d, out_d = aps
    r = f"r{rep}"

    # DMA order matters: wq lands first (it feeds the PE warm-up matmuls),
    # then xT which gates the first projection matmuls.
    wsbs = {}
    wsbs["wq"] = persist.tile([128, ND, JL], BF16, tag="wq", name=f"wq_{r}")
    nc.sync.dma_start(wsbs["wq"][:], wq_d.rearrange("(t p) j -> p t j", p=128))
    xsb = persist.tile([128, ND, S], BF16, tag="xT", name=f"xT_{r}")
    xT_r = xT_d.rearrange("(t p) s -> p t s", p=128)
    for di in range(ND):
        nc.sync.dma_start(xsb[:, di, :], xT_r[:, di, :])
    for nm, d_ap in (("wk", wk_d), ("wv", wv_d)):
        wsbs[nm] = persist.tile([128, ND, JL], BF16, tag=nm, name=f"{nm}_{r}")
        nc.sync.dma_start(wsbs[nm][:], d_ap.rearrange("(t p) j -> p t j", p=128))
    c2 = persist.tile([128, S], BF16, tag="c2", name=f"c2_{r}")
    s2 = persist.tile([128, S], BF16, tag="s2", name=f"s2_{r}")
    nc.sync.dma_start(c2[:], c2_d)
    nc.sync.dma_start(s2[:], s2_d)
    wo_sb = persist.tile([128, 2, D], BF16, tag="wo", name=f"wo_{r}")
    nc.sync.dma_start(wo_sb[:], wo_d.rearrange("(t p) n -> p t n", p=128))
    ident = persist.tile([128, 128], BF16, tag="ident", name=f"ident_{r}")
    make_identity(nc, ident[:])

    qTr = persist.tile([128, 2, S], BF16, tag="qTr", name=f"qTr_{r}")
    kTr = persist.tile([128, 2, S], BF16, tag="kTr", name=f"kTr_{r}")
    vON = persist.tile([128, NSK, NHL * 65], BF16, tag="vON", name=f"vON_{r}")
    attn = persist.tile([128, NM, JL], BF16, tag="attn", name=f"attn_{r}")
    attnT = persist.tile([128, 2, S], BF16, tag="attnT", name=f"attnT_{r}")

    vON_r = vON[:].rearrange("p t (h c) -> p (t h) c", c=65)
    nc.vector.memset(vON_r[:, :, 64:65], 1.0)

    def emit_qk_tile(pool, wsb, dst, jt, st, fast_rope=False):
        """One [128, 512] q/k projection tile + RoPE into dst (qTr/kTr)."""
        sl = slice(st * 512, (st + 1) * 512)
        ps = pool.tile([128, 512], F32, tag="ps1", name=f"pqk_{r}")
        for di in range(ND):
            nc.tensor.matmul(
                ps[:],
                lhsT=wsb[:, di, jt * 128:(jt + 1) * 128],
                rhs=xsb[:, di, sl],
                start=(di == 0),
                stop=(di == ND - 1),
            )
        # RoPE in bf16 (fast DVE mode). rot[p] = q[p]*cos - (q*s2)[swap(p)]
        # where swap flips the a/b 32-row halves within each head; the
        # partition swap rides on GpSimd (single-input shifted copies).
        qb = rope.tile([128, 512], BF16, tag="qb", name=f"qb_{r}")
        nc.vector.tensor_copy(qb[:], ps[:])
        w2 = rope.tile([128, 512], BF16, tag="w2", name=f"w2_{r}")
        nc.vector.tensor_mul(w2[:], qb[:], s2[:, sl])
        u = rope.tile([128, 512], BF16, tag="u", name=f"u_{r}")
        for blk in range(4):
            o = blk * 32
            so = o ^ 32
            eng = nc.vector if (fast_rope or blk >= 2) else nc.gpsimd
            eng.tensor_copy(u[o:o + 32, :], w2[so:so + 32, :])
        t2 = rope.tile([128, 512], BF16, tag="t2", name=f"t2_{r}")
        nc.vector.tensor_mul(t2[:], qb[:], c2[:, sl])
        nc.vector.tensor_sub(dst[:, jt, sl], t2[:], u[:])

    def emit_v_tile(pool, st):
        psv = pool.tile([128, JL], F32, tag="ps1", name=f"pv_{r}")
        for di in range(ND):
            nc.tensor.matmul(
                psv[:],
                lhsT=xsb[:, di, st * 128:(st + 1) * 128],
                rhs=wsbs["wv"][:, di, :],
                start=(di == 0),
                stop=(di == ND - 1),
            )
        nc.vector.tensor_copy(
            vON_r[:, st * NHL:(st + 1) * NHL, 0:64],
            psv[:].rearrange("p (h e) -> p h e", e=64),
        )

    # ---------------- attention (projections stream through the same pool) --
    with (
        tc.tile_pool(name=f"sc_{r}", bufs=2, space="PSUM") as scp,
        tc.tile_pool(name=f"ps1_{r}", bufs=2, space="PSUM") as ps1p,
        tc.tile_pool(name=f"expp_{r}", bufs=3) as expp,
        tc.tile_pool(name=f"normp_{r}", bufs=8) as normp,
        tc.tile_pool(name=f"osb_{r}", bufs=8) as osbp,
    ):
        # PE warm-up: the HAM clock gate keeps a cold PE at half rate for the
        # first ~3.4us of activity. The xT load takes ~13us during which PE
        # would idle cold; run throwaway matmuls on the (early-arriving) wq
        # tile so the real projections start at full clock.
        warm = scp.tile([128, 3, 512], F32, tag="sc", name=f"warm_{r}")
        for i in range(56):
            nc.tensor.matmul(
                warm[:, i % 3, 0:256],
                lhsT=wsbs["wq"][:, 0, 0:128],
                rhs=wsbs["wq"][:, 0, 0:256],
                start=True, stop=True,
            )
        # prefetch the ScalarE activation-table load (~1.3us) into the DMA
        # window so the first real exp doesn't pay it.
        twarm = normp.tile([128, 2], F32, tag="rec", name=f"twarm_{r}")
        nc.scalar.activation(out=twarm[:, :], in_=ident[:, 0:2], func=AF.Exp)

        # phase 1: only the projections gating the very first score batch —
        # PE executes in emission order, so everything else streams between
        # score/exp groups (each projection must still be emitted before the
        # first matmul that reads it).
        emit_qk_tile(ps1p, wsbs["wq"], qTr, 0, 0, fast_rope=True)
        emit_qk_tile(ps1p, wsbs["wk"], kTr, 0, 0, fast_rope=True)
        emit_qk_tile(ps1p, wsbs["wk"], kTr, 0, 1, fast_rope=True)
        def emit_scores(hp, c, n, exp_tiles):
            et = expp.tile([128, 16, 512], BF16, tag="expT", name=f"expT_{r}")
            exp_tiles[n] = et
            for start, size in SC_BATCHES:
                sc = scp.tile([128, size, 512], F32, tag="sc", name=f"sc_{r}")
                for k in range(size):
                    slot = start + k
                    t2_, h = slot // 2, slot % 2
                    tg = c * TPC + t2_
                    nc.tensor.matmul(
                        sc[:, k, :],
                        lhsT=kTr[64 * h:64 * (h + 1), hp, tg * 128:(tg + 1) * 128],
                        rhs=qTr[64 * h:64 * (h + 1), hp, n * 512:(n + 1) * 512],
                        start=True, stop=True,
                    )
                nc.scalar.activation(
                    out=et[:, start:start + size, :],
                    in_=sc[:, 0:size, :],
                    func=AF.Exp,
                )

        def emit_wv(hp, c, n, exp_tiles):
            et = exp_tiles.pop(n)
            for m2 in range(0, 4, 2):
                # two m-tiles x two heads packed in one PSUM bank:
                # (m-rel j, head h) at cols [132j + 66h, +65). Only the very
                # first matmul uses start=True (whole-bank has_written clear);
                # later groups overwrite-then-accumulate per element.
                psB = ps1p.tile([128, 264], F32, tag="ps1", name=f"psB_{r}")
                psBr = psB[:].rearrange("p (j h e) -> p j h e", h=2, e=66)
                for j in range(2):
                    for h in range(2):
                        hl = hp * 2 + h
                        for t2_ in range(TPC):
                            tg = c * TPC + t2_
                            off = j * 132 + h * 66
                            nc.tensor.matmul(
                                psB[:, off:off + 65],
                                lhsT=et[:, 2 * t2_ + h, (m2 + j) * 128:(m2 + j + 1) * 128],
                                rhs=vON[:, tg, hl * 65:(hl + 1) * 65],
                                start=(j == 0 and h == 0 and t2_ == 0),
                                stop=(j == 1 and h == 1 and t2_ == TPC - 1),
                                skip_group_check=True,
                            )
                rec = normp.tile([128, 2, 2], F32, tag="rec", name=f"rec_{r}")
                nc.vector.reciprocal(rec[:, :, :], psBr[:, :, :, 64:65])
                for j in range(2):
                    m = n * 4 + m2 + j
                    for h in range(2):
                        hl = hp * 2 + h
                        dstp = attn[:, m, hl * 64:(hl + 1) * 64]
                        if c == 0:
                            nc.vector.tensor_scalar_mul(dstp, psBr[:, j, h, 0:64], rec[:, j, h:h + 1])
                        else:
                            nc.vector.scalar_tensor_tensor(
                                out=dstp, in0=psBr[:, j, h, 0:64], scalar=rec[:, j, h:h + 1],
                                in1=dstp, op0=OP.mult, op1=OP.add,
                            )

        def emit_out_quarter(n):
            """Transpose + output projection for sq tiles m in quarter n.
            Quarters 2-3 are emitted after the last score batch, so they can
            use the (then idle) score pool's banks instead of contending with
            the W@V accumulators for the shared 1-bank pool."""
            pool, tag = (scp, "sc") if n >= 2 else (ps1p, "ps1")
            for m in range(n * 4, n * 4 + 4):
                for jt in range(2):
                    pstile = pool.tile([128, 128], BF16, tag=tag, name=f"pt_{r}")
                    nc.tensor.transpose(pstile[:], attn[:, m, jt * 128:(jt + 1) * 128], ident[:])
                    nc.vector.tensor_copy(attnT[:, jt, m * 128:(m + 1) * 128], pstile[:])
            for m in range(n * 4, n * 4 + 4):
                for nn in range(2):
                    pso = pool.tile([128, 512], F32, tag=tag, name=f"po_{r}")
                    for jt in range(2):
                        nc.tensor.matmul(
                            pso[:],
                            lhsT=attnT[:, jt, m * 128:(m + 1) * 128],
                            rhs=wo_sb[:, jt, nn * 512:(nn + 1) * 512],
                            start=(jt == 0),
                            stop=(jt == 1),
                        )
                    osb = osbp.tile([128, 512], BF16, tag="osb", name=f"osb_{r}")
                    if n >= 2 and (m + nn) % 2 == 0:
                        nc.scalar.copy(osb[:], pso[:])
                    else:
                        nc.vector.tensor_copy(osb[:], pso[:])
                    nc.sync.dma_start(out_d[m * 128:(m + 1) * 128, nn * 512:(nn + 1) * 512], osb[:])

        # per-(hp, c) list of "extra" work items interleaved between exp
        # batches: v-projection tiles, deferred jt1 projections, fused
        # output-projection quarters. Slot positions: after wv(0..3).
        def qk(w, dst, jt, st):
            return lambda: emit_qk_tile(ps1p, wsbs[w], dst, jt, st)

        def vt(st):
            return lambda: emit_v_tile(ps1p, st)

        # extras[(hp, c)] = 8 slots of deferred work interleaved at:
        # [before s1, before s2-pre, after wv0, before s3, after wv1,
        #  after s3-emit/wv2, after wv3-a, after wv3-b]
        extras = {
            (0, 0): [
                [qk("wq", qTr, 0, 1)],                        # before scores n=1
                [qk("wq", qTr, 0, 2)],                        # after wv0
                [qk("wq", qTr, 0, 3), qk("wk", kTr, 0, 2)],   # after wv1
                [qk("wk", kTr, 0, 3)],                        # after wv2
                [],
            ],
            (0, 1): [
                [qk("wq", qTr, 1, 0)],
                [qk("wq", qTr, 1, 1)],
                [qk("wk", kTr, 1, 0)],
                [qk("wk", kTr, 1, 1)],
                [],
            ],
            (1, 0): [
                [qk("wq", qTr, 1, 2)],
                [qk("wq", qTr, 1, 3)],
                [qk("wk", kTr, 1, 2)],
                [qk("wk", kTr, 1, 3)],
                [],
            ],
            (1, 1): [[], [], [], [], []],
        }

        for hp in range(2):
            for c in range(2):
                last = (hp == 1 and c == 1)
                ext = extras[(hp, c)]
                ex = {}
                # v tiles needed by this (hp, c)'s wv: chunk tiles c*8..c*8+8
                pre = [vt(c * TPC + i) for i in range(TPC)] if hp == 0 else []
                emit_scores(hp, c, 0, ex)
                for f in ext[0]:
                    f()
                emit_scores(hp, c, 1, ex)
                for f in pre:
                    f()
                emit_wv(hp, c, 0, ex)
                for f in ext[1]:
                    f()
                if last:
                    emit_out_quarter(0)
                emit_scores(hp, c, 2, ex)
                emit_wv(hp, c, 1, ex)
                for f in ext[2]:
                    f()
                if last:
                    emit_out_quarter(1)
                emit_scores(hp, c, 3, ex)
                emit_wv(hp, c, 2, ex)
                for f in ext[3]:
                    f()
                if last:
                    emit_out_quarter(2)
                emit_wv(hp, c, 3, ex)
                for f in ext[4]:
                    f()
                if last:
                    emit_out_quarter(3)


def _build_nc(reps=1):
    nc = bacc.Bacc("TRN2", target_bir_lowering=False, debug=False, num_devices=8)

    aps = (
        nc.dram_tensor("xT", [D, S], BF16, kind="ExternalInput").ap(),
        nc.dram_tensor("wq", [D, JL], BF16, kind="ExternalInput").ap(),
        nc.dram_tensor("wk", [D, JL], BF16, kind="ExternalInput").ap(),
        nc.dram_tensor("wv", [D, JL], BF16, kind="ExternalInput").ap(),
        nc.dram_tensor("wo", [JL, D], BF16, kind="ExternalInput").ap(),
        nc.dram_tensor("c2", [128, S], BF16, kind="ExternalInput").ap(),
        nc.dram_tensor("s2", [128, S], BF16, kind="ExternalInput").ap(),
        nc.dram_tensor("out", [S, D], BF16, kind="ExternalOutput").ap(),
    )

    with (
        tile.TileContext(nc) as tc,
        tc.tile_pool(name="persist", bufs=1) as persist,
        tc.tile_pool(name="rope", bufs=3) as rope,
    ):
        for rep in range(reps):
            _emit_body(nc, tc, persist, rope, aps, rep)

    nc.compile()
    return nc


def _get_nc(reps=1):
    if reps not in _CACHED:
        _CACHED[reps] = _build_nc(reps)
    return _CACHED[reps]


def _host_prep(hidden_states, freqs_cis, Wq, Wk, Wv, Wo):
    bf16 = ml_dtypes.bfloat16
    hs = np.asarray(hidden_states, dtype=np.float32)
    fc = np.asarray(freqs_cis, dtype=np.float32)
    Wq = np.asarray(Wq, dtype=np.float32)
    Wk = np.asarray(Wk, dtype=np.float32)
    Wv = np.asarray(Wv, dtype=np.float32)
    Wo = np.asarray(Wo, dtype=np.float32)

    cos, sin = fc[:, :, 0], fc[:, :, 1]                      # [S, 32]
    i_idx = np.arange(128) % 32
    sign = np.where((np.arange(128) % 64) < 32, -1.0, 1.0).astype(np.float32)
    c2 = np.ascontiguousarray(cos.T[i_idx]).astype(bf16)     # [128, S]
    s2 = np.ascontiguousarray(sin.T[i_idx] * sign[:, None]).astype(bf16)

    xTs = [np.ascontiguousarray(hs[b].T).astype(bf16) for b in range(B)]

    in_maps = []
    for core in range(8):
        b, g = core // 4, core % 4
        perm = []
        for h in range(4 * g, 4 * g + 4):
            perm += [h * 64 + 2 * i for i in range(32)]
            perm += [h * 64 + 2 * i + 1 for i in range(32)]
        perm = np.array(perm)
        jcols = slice(g * JL, (g + 1) * JL)
        in_maps.append({
            "xT": xTs[b],
            "wq": np.ascontiguousarray(Wq[:, perm] * (HD ** -0.5)).astype(bf16),
            "wk": np.ascontiguousarray(Wk[:, perm]).astype(bf16),
            "wv": np.ascontiguousarray(Wv[:, jcols]).astype(bf16),
            "wo": np.ascontiguousarray(Wo[jcols, :]).astype(bf16),
            "c2": c2,
            "s2": s2,
        })
    return in_maps


def kernel(hidden_states, freqs_cis, Wq, Wk, Wv, Wo, _trace=False, _reps=1):
    nc = _get_nc(_reps)
    in_maps = _host_prep(hidden_states, freqs_cis, Wq, Wk, Wv, Wo)
    if _trace:
        try:
            from antenv.axon_hooks import get_axon_ntff_profile_hook  # noqa: F401
        except ImportError:
            _trace = False
    res = run_bass_kernel_spmd(nc, in_maps, core_ids=list(range(8)), trace=_trace)
    outs = [r["out"].astype(np.float32) for r in res.results]
    full = np.zeros((B, S, D), dtype=np.float32)
    for core in range(8):
        full[core // 4] += outs[core]
    if _trace:
        kernel._last_results = res
    return full


# revision 41
# speedup vs baseline: 1726.4775x; 1.0161x over previous
"""Trainium2 Bass kernel for chunked flash-attention block (B=2, S=2048, D=1024, H=16).

Sharding: 8 cores = 2 batches x 4 head-groups (4 heads each). Each core computes
its heads' QKV projections + RoPE + per-chunk-softmax attention + its slice of the
output projection; the host sums the 4 partial out-projections per batch.

All device-side activations are kept transposed ([feature, seq]) so every matmul
contraction lands on the partition dimension with no on-device transposes of the
inputs. Emission order streams the attention (exp on ScalarE is the bottleneck):
head-pair 0 attention starts right after its projections; head-pair 1 projections
are interleaved into head-pair 0's attention; the output projection is fused into
the last attention pass.
"""

import numpy as np
import ml_dtypes

import concourse.bass as bass
import concourse.tile as tile
from concourse import bacc, mybir
from concourse.bass_utils import run_bass_kernel_spmd
from concourse.masks import make_identity

dt = mybir.dt
F32 = dt.float32
BF16 = dt.bfloat16
AF = mybir.ActivationFunctionType
OP = mybir.AluOpType

B, S, D, H, HD = 2, 2048, 1024, 16, 64
CHUNK = 1024
NHL = 4              # local heads per core
JL = NHL * HD        # 256 local projected dims
ND = D // 128        # 8 k-tiles for the projections
NSQ = S // 512       # 4 sq n-tiles
NSK = S // 128       # 16 sk p-tiles
NCH = S // CHUNK     # 2 key chunks
TPC = CHUNK // 128   # 8 sk tiles per chunk
NM = S // 128        # 16 sq p-tiles

# exp batches per (hp, chunk, n): 16 (t, h) score slots -> 6 ACT instructions
SC_BATCHES = ((0, 2), (2, 3), (5, 3), (8, 3), (11, 3), (14, 2))

_CACHED = {}


def _emit_body(nc, tc, persist, rope, aps, rep):
    """Emit one full iteration of the kernel into the open TileContext."""
    xT_d, wq_d, wk_d, wv_d, wo_d, c2_d, s2_d, out_d = aps
    r = f"r{rep}"

    # DMA order matters: wq lands first (it feeds the PE warm-up matmuls),
    # then xT which gates the first projection matmuls.
    wsbs = {}
    wsbs["wq"] = persist.tile([128, ND, JL], BF16, tag="wq", name=f"wq_{r}")
    nc.sync.dma_start(wsbs["wq"][:], wq_d.rearrange("(t p) j -> p t j", p=128))
    xsb = persist.tile([128, ND, S], BF16, tag="xT", name=f"xT_{r}")
    xT_r = xT_d.rearrange("(t p) s -> p t s", p=128)
    for di in range(ND):
        nc.sync.dma_start(xsb[:, di, :], xT_r[:, di, :])
    for nm, d_ap in (("wk", wk_d), ("wv", wv_d)):
        wsbs[nm] = persist.tile([128, ND, JL], BF16, tag=nm, name=f"{nm}_{r}")
        nc.sync.dma_start(wsbs[nm][:], d_ap.rearrange("(t p) j -> p t j", p=128))
    c2 = persist.tile([128, S], BF16, tag="c2", name=f"c2_{r}")
    s2 = persist.tile([128, S], BF16, tag="s2", name=f"s2_{r}")
    nc.sync.dma_start(c2[:], c2_d)
    nc.sync.dma_start(s2[:], s2_d)
    wo_sb = persist.tile([128, 2, D], BF16, tag="wo", name=f"wo_{r}")
    nc.sync.dma_start(wo_sb[:], wo_d.rearrange("(t p) n -> p t n", p=128))
    ident = persist.tile([128, 128], BF16, tag="ident", name=f"ident_{r}")
    make_identity(nc, ident[:])

    qTr = persist.tile([128, 2, S], BF16, tag="qTr", name=f"qTr_{r}")
    kTr = persist.tile([128, 2, S], BF16, tag="kTr", name=f"kTr_{r}")
    vON = persist.tile([128, NSK, NHL * 65], BF16, tag="vON", name=f"vON_{r}")
    attn = persist.tile([128, NM, JL], BF16, tag="attn", name=f"attn_{r}")
    attnT = persist.tile([128, 2, S], BF16, tag="attnT", name=f"attnT_{r}")

    vON_r = vON[:].rearrange("p t (h c) -> p (t h) c", c=65)
    nc.vector.memset(vON_r[:, :, 64:65], 1.0)

    def emit_qk_tile(pool, wsb, dst, jt, st, fast_rope=False):
        """One [128, 512] q/k projection tile + RoPE into dst (qTr/kTr)."""
        sl = slice(st * 512, (st + 1) * 512)
        ps = pool.tile([128, 512], F32, tag="ps1", name=f"pqk_{r}")
        for di in range(ND):
            nc.tensor.matmul(
                ps[:],
                lhsT=wsb[:, di, jt * 128:(jt + 1) * 128],
                rhs=xsb[:, di, sl],
                start=(di == 0),
                stop=(di == ND - 1),
            )
        # RoPE in bf16 (fast DVE mode). rot[p] = q[p]*cos - (q*s2)[swap(p)]
        # where swap flips the a/b 32-row halves within each head; the
        # partition swap rides on GpSimd (single-input shifted copies).
        qb = rope.tile([128, 512], BF16, tag="qb", name=f"qb_{r}")
        nc.vector.tensor_copy(qb[:], ps[:])
        w2 = rope.tile([128, 512], BF16, tag="w2", name=f"w2_{r}")
        nc.vector.tensor_mul(w2[:], qb[:], s2[:, sl])
        u = rope.tile([128, 512], BF16, tag="u", name=f"u_{r}")
        for blk in range(4):
            o = blk * 32
            so = o ^ 32
            nc.vector.tensor_copy(u[o:o + 32, :], w2[so:so + 32, :])
        t2 = rope.tile([128, 512], BF16, tag="t2", name=f"t2_{r}")
        nc.vector.tensor_mul(t2[:], qb[:], c2[:, sl])
        nc.vector.tensor_sub(dst[:, jt, sl], t2[:], u[:])

    def emit_v_tile(pool, st):
        psv = pool.tile([128, JL], F32, tag="ps1", name=f"pv_{r}")
        for di in range(ND):
            nc.tensor.matmul(
                psv[:],
                lhsT=xsb[:, di, st * 128:(st + 1) * 128],
                rhs=wsbs["wv"][:, di, :],
                start=(di == 0),
                stop=(di == ND - 1),
            )
        nc.vector.tensor_copy(
            vON_r[:, st * NHL:(st + 1) * NHL, 0:64],
            psv[:].rearrange("p (h e) -> p h e", e=64),
        )

    # ---------------- attention (projections stream through the same pool) --
    with (
        tc.tile_pool(name=f"sc_{r}", bufs=2, space="PSUM") as scp,
        tc.tile_pool(name=f"ps1_{r}", bufs=2, space="PSUM") as ps1p,
        tc.tile_pool(name=f"expp_{r}", bufs=3) as expp,
        tc.tile_pool(name=f"normp_{r}", bufs=8) as normp,
        tc.tile_pool(name=f"osb_{r}", bufs=8) as osbp,
    ):
        # PE warm-up: the HAM clock gate keeps a cold PE at half rate for the
        # first ~3.4us of activity. The xT load takes ~13us during which PE
        # would idle cold; run throwaway matmuls on the (early-arriving) wq
        # tile so the real projections start at full clock.
        warm = scp.tile([128, 3, 512], F32, tag="sc", name=f"warm_{r}")
        for i in range(56):
            nc.tensor.matmul(
                warm[:, i % 3, 0:256],
                lhsT=wsbs["wq"][:, 0, 0:128],
                rhs=wsbs["wq"][:, 0, 0:256],
                start=True, stop=True,
            )
        # prefetch the ScalarE activation-table load (~1.3us) into the DMA
        # window so the first real exp doesn't pay it.
        twarm = normp.tile([128, 2], F32, tag="rec", name=f"twarm_{r}")
        nc.scalar.activation(out=twarm[:, :], in_=ident[:, 0:2], func=AF.Exp)

        # phase 1: only the projections gating the very first score batch —
        # PE executes in emission order, so everything else streams between
        # score/exp groups (each projection must still be emitted before the
        # first matmul that reads it).
        emit_qk_tile(ps1p, wsbs["wq"], qTr, 0, 0, fast_rope=True)
        emit_qk_tile(ps1p, wsbs["wk"], kTr, 0, 0, fast_rope=True)
        emit_qk_tile(ps1p, wsbs["wk"], kTr, 0, 1, fast_rope=True)
        def emit_scores(hp, c, n, exp_tiles):
            et = expp.tile([128, 16, 512], BF16, tag="expT", name=f"expT_{r}")
            exp_tiles[n] = et
            for start, size in SC_BATCHES:
                sc = scp.tile([128, size, 512], F32, tag="sc", name=f"sc_{r}")
                for k in range(size):
                    slot = start + k
                    t2_, h = slot // 2, slot % 2
                    tg = c * TPC + t2_
                    nc.tensor.matmul(
                        sc[:, k, :],
                        lhsT=kTr[64 * h:64 * (h + 1), hp, tg * 128:(tg + 1) * 128],
                        rhs=qTr[64 * h:64 * (h + 1), hp, n * 512:(n + 1) * 512],
                        start=True, stop=True,
                    )
                nc.scalar.activation(
                    out=et[:, start:start + size, :],
                    in_=sc[:, 0:size, :],
                    func=AF.Exp,
                )

        def emit_wv(hp, c, n, exp_tiles):
            et = exp_tiles.pop(n)
            for m2 in range(0, 4, 2):
                # two m-tiles x two heads packed in one PSUM bank:
                # (m-rel j, head h) at cols [132j + 66h, +65). Only the very
                # first matmul uses start=True (whole-bank has_written clear);
                # later groups overwrite-then-accumulate per element.
                psB = ps1p.tile([128, 264], F32, tag="ps1", name=f"psB_{r}")
                psBr = psB[:].rearrange("p (j h e) -> p j h e", h=2, e=66)
                for j in range(2):
                    for h in range(2):
                        hl = hp * 2 + h
                        for t2_ in range(TPC):
                            tg = c * TPC + t2_
                            off = j * 132 + h * 66
                            nc.tensor.matmul(
                                psB[:, off:off + 65],
                                lhsT=et[:, 2 * t2_ + h, (m2 + j) * 128:(m2 + j + 1) * 128],
                                rhs=vON[:, tg, hl * 65:(hl + 1) * 65],
                                start=(j == 0 and h == 0 and t2_ == 0),
                                stop=(j == 1 and h == 1 and t2_ == TPC - 1),
                                skip_group_check=True,
                            )
                rec = normp.tile([128, 2, 2], F32, tag="rec", name=f"rec_{r}")
                nc.vector.reciprocal(rec[:, :, :], psBr[:, :, :, 64:65])
                for j in range(2):
                    m = n * 4 + m2 + j
                    for h in range(2):
                        hl = hp * 2 + h
                        dstp = attn[:, m, hl * 64:(hl + 1) * 64]
                        if c == 0:
                            nc.vector.tensor_scalar_mul(dstp, psBr[:, j, h, 0:64], rec[:, j, h:h + 1])
                        else:
                            nc.vector.scalar_tensor_tensor(
                                out=dstp, in0=psBr[:, j, h, 0:64], scalar=rec[:, j, h:h + 1],
                                in1=dstp, op0=OP.mult, op1=OP.add,
                            )

        def emit_out_quarter(n):
            """Transpose + output projection for sq tiles m in quarter n.
            Quarters 2-3 are emitted after the last score batch, so they can
            use the (then idle) score pool's banks instead of contending with
            the W@V accumulators for the shared 1-bank pool."""
            pool, tag = (scp, "sc") if n >= 2 else (ps1p, "ps1")
            for m in range(n * 4, n * 4 + 4):
                for jt in range(2):
                    pstile = pool.tile([128, 128], BF16, tag=tag, name=f"pt_{r}")
                    nc.tensor.transpose(pstile[:], attn[:, m, jt * 128:(jt + 1) * 128], ident[:])
                    nc.vector.tensor_copy(attnT[:, jt, m * 128:(m + 1) * 128], pstile[:])
            for m in range(n * 4, n * 4 + 4):
                for nn in range(2):
                    pso = pool.tile([128, 512], F32, tag=tag, name=f"po_{r}")
                    for jt in range(2):
                        nc.tensor.matmul(
                            pso[:],
                            lhsT=attnT[:, jt, m * 128:(m + 1) * 128],
                            rhs=wo_sb[:, jt, nn * 512:(nn + 1) * 512],
                            start=(jt == 0),
                            stop=(jt == 1),
                        )
                    osb = osbp.tile([128, 512], BF16, tag="osb", name=f"osb_{r}")
                    if n >= 2 and (m + nn) % 2 == 0:
                        nc.scalar.copy(osb[:], pso[:])
                    else:
                        nc.vector.tensor_copy(osb[:], pso[:])
                    nc.sync.dma_start(out_d[m * 128:(m + 1) * 128, nn * 512:(nn + 1) * 512], osb[:])

        # per-(hp, c) list of "extra" work items interleaved between exp
        # batches: v-projection tiles, deferred jt1 projections, fused
        # output-projection quarters. Slot positions: after wv(0..3).
        def qk(w, dst, jt, st):
            return lambda: emit_qk_tile(ps1p, wsbs[w], dst, jt, st)

        def vt(st):
            return lambda: emit_v_tile(ps1p, st)

        # extras[(hp, c)] = 8 slots of deferred work interleaved at:
        # [before s1, before s2-pre, after wv0, before s3, after wv1,
        #  after s3-emit/wv2, after wv3-a, after wv3-b]
        extras = {
            (0, 0): [
                [qk("wq", qTr, 0, 1)],                        # before scores n=1
                [qk("wq", qTr, 0, 2)],                        # after wv0
                [qk("wq", qTr, 0, 3), qk("wk", kTr, 0, 2)],   # after wv1
                [qk("wk", kTr, 0, 3)],                        # after wv2
                [],
            ],
            (0, 1): [
                [qk("wq", qTr, 1, 0)],
                [qk("wq", qTr, 1, 1)],
                [qk("wk", kTr, 1, 0)],
                [qk("wk", kTr, 1, 1)],
                [],
            ],
            (1, 0): [
                [qk("wq", qTr, 1, 2)],
                [qk("wq", qTr, 1, 3)],
                [qk("wk", kTr, 1, 2)],
                [qk("wk", kTr, 1, 3)],
                [],
            ],
            (1, 1): [[], [], [], [], []],
        }

        for hp in range(2):
            for c in range(2):
                last = (hp == 1 and c == 1)
                ext = extras[(hp, c)]
                ex = {}
                # v tiles needed by this (hp, c)'s wv: chunk tiles c*8..c*8+8
                pre = [vt(c * TPC + i) for i in range(TPC)] if hp == 0 else []
                emit_scores(hp, c, 0, ex)
                for f in ext[0]:
                    f()
                emit_scores(hp, c, 1, ex)
                for f in pre:
                    f()
                emit_wv(hp, c, 0, ex)
                for f in ext[1]:
                    f()
                if last:
                    emit_out_quarter(0)
                emit_scores(hp, c, 2, ex)
                emit_wv(hp, c, 1, ex)
                for f in ext[2]:
                    f()
                if last:
                    emit_out_quarter(1)
                emit_scores(hp, c, 3, ex)
                emit_wv(hp, c, 2, ex)
                for f in ext[3]:
                    f()
                if last:
                    emit_out_quarter(2)
                emit_wv(hp, c, 3, ex)
                for f in ext[4]:
                    f()
                if last:
                    emit_out_quarter(3)


def _build_nc(reps=1):
    nc = bacc.Bacc("TRN2", target_bir_lowering=False, debug=False, num_devices=8)

    aps = (
        nc.dram_tensor("xT", [D, S], BF16, kind="ExternalInput").ap(),
        nc.dram_tensor("wq", [D, JL], BF16, kind="ExternalInput").ap(),
        nc.dram_tensor("wk", [D, JL], BF16, kind="ExternalInput").ap(),
        nc.dram_tensor("wv", [D, JL], BF16, kind="ExternalInput").ap(),
        nc.dram_tensor("wo", [JL, D], BF16, kind="ExternalInput").ap(),
        nc.dram_tensor("c2", [128, S], BF16, kind="ExternalInput").ap(),
        nc.dram_tensor("s2", [128, S], BF16, kind="ExternalInput").ap(),
        nc.dram_tensor("out", [S, D], BF16, kind="ExternalOutput").ap(),
    )

    with (
        tile.TileContext(nc) as tc,
        tc.tile_pool(name="persist", bufs=1) as persist,
        tc.tile_pool(name="rope", bufs=3) as rope,
    ):
        for rep in range(reps):
            _emit_body(nc, tc, persist, rope, aps, rep)

    nc.compile()
    return nc


def _get_nc(reps=1):
    if reps not in _CACHED:
        _CACHED[reps] = _build_nc(reps)
    return _CACHED[reps]


def _host_prep(hidden_states, freqs_cis, Wq, Wk, Wv, Wo):
    bf16 = ml_dtypes.bfloat16
    hs = np.asarray(hidden_states, dtype=np.float32)
    fc = np.asarray(freqs_cis, dtype=np.float32)
    Wq = np.asarray(Wq, dtype=np.float32)
    Wk = np.asarray(Wk, dtype=np.float32)
    Wv = np.asarray(Wv, dtype=np.float32)
    Wo = np.asarray(Wo, dtype=np.float32)

    cos, sin = fc[:, :, 0], fc[:, :, 1]                      # [S, 32]
    i_idx = np.arange(128) % 32
    sign = np.where((np.arange(128) % 64) < 32, -1.0, 1.0).astype(np.float32)
    c2 = np.ascontiguousarray(cos.T[i_idx]).astype(bf16)     # [128, S]
    s2 = np.ascontiguousarray(sin.T[i_idx] * sign[:, None]).astype(bf16)

    xTs = [np.ascontiguousarray(hs[b].T).astype(bf16) for b in range(B)]

    in_maps = []
    for core in range(8):
        b, g = core // 4, core % 4
        perm = []
        for h in range(4 * g, 4 * g + 4):
            perm += [h * 64 + 2 * i for i in range(32)]
            perm += [h * 64 + 2 * i + 1 for i in range(32)]
        perm = np.array(perm)
        jcols = slice(g * JL, (g + 1) * JL)
        in_maps.append({
            "xT": xTs[b],
            "wq": np.ascontiguousarray(Wq[:, perm] * (HD ** -0.5)).astype(bf16),
            "wk": np.ascontiguousarray(Wk[:, perm]).astype(bf16),
            "wv": np.ascontiguousarray(Wv[:, jcols]).astype(bf16),
            "wo": np.ascontiguousarray(Wo[jcols, :]).astype(bf16),
            "c2": c2,
            "s2": s2,
        })
    return in_maps


def kernel(hidden_states, freqs_cis, Wq, Wk, Wv, Wo, _trace=False, _reps=1):
    nc = _get_nc(_reps)
    in_maps = _host_prep(hidden_states, freqs_cis, Wq, Wk, Wv, Wo)
    if _trace:
        try:
            from antenv.axon_hooks import get_axon_ntff_profile_hook  # noqa: F401
        except ImportError:
            _trace = False
    res = run_bass_kernel_spmd(nc, in_maps, core_ids=list(range(8)), trace=_trace)
    outs = [r["out"].astype(np.float32) for r in res.results]
    full = np.zeros((B, S, D), dtype=np.float32)
    for core in range(8):
        full[core // 4] += outs[core]
    if _trace:
        kernel._last_results = res
    return full
